# revision 33
# baseline (speedup 1.0000x reference)
"""EnsembleFC (E=16 MLPs, 512->512->512->1, relu) on 8 TRN2 NeuronCores.

Strategy (expert parallel): each core owns E/8 = 2 ensemble members' weights
and computes their [B] output column; x is replicated. All activations stay
in "feature-major" (transposed) layout so no on-device transposes are needed:

    h1^T = relu(W1^T @ x^T + b1)      [H, B]
    h2^T = relu(W2^T @ h1^T + b2)     [H, B]
    out^T = W3^T @ h2^T               [1, B]   (b3 added on host)

Matmuls run in float32r (TRN2 reduced-precision fp32 PE mode, 1 cycle/row —
4x faster than plain fp32, ~20x more accurate than bf16; measured scaled
error ~1.5e-4 per 128-deep contraction).

Raw Bass (no Tile framework): this container's walrus rejects instructions
with more than a couple of sync waits, which Tile's auto-generated drains
exceed. Explicit per-engine programs with standalone waits keep every
instruction at one wait.

Pipeline layout per chunk of 512 batch columns:
  PE:  L1(m0) L1(m1) L2(m0) L2(m1) L3(m0) L3(m1)  -- member interleave hides
       the relu latency between a member's L1 and L2.
  PSUM: each member-layer pair owns 2 banks (mt % 2 rotation); L3 reuses the
       member's first L2 bank at partition 0.
  ACT: relu+bias drains psum into h1/h2 (f32r).
  DVE: copies L3 psum rows to the output staging buffer.
  SP:  weight DMAs (per-tensor sems, split per k-tile), x chunk DMAs
       (per-slot sems -- DMA queue completions are unordered), output stores.
A short burst of dummy matmuls on scratch SBUF during the DMA prologue keeps
the PE HAM clock-gate warm so chunk 0 runs at full clock.
"""
import numpy as np

E, D, H, B = 16, 512, 512, 8192
N_CORES = 8
MPC = E // N_CORES          # members per core
KT = D // 128               # k-tiles per 512 contraction
MT = H // 128               # m-tiles per 512 output dim
CH = 512                    # batch columns per chunk (one psum bank)
NCH = B // CH               # chunks
XBUF = 4                    # x chunk buffering

_CACHE = {}


def _build():
    import concourse.bass as bass
    from concourse import mybir

    f32 = mybir.dt.float32
    f32r = mybir.dt.float32r

    nc = bass.Bass("TRN2", target_bir_lowering=False, debug=False,
                   num_devices=N_CORES)

    xT = nc.dram_tensor("xT", [D, B], f32r, kind="ExternalInput").ap()
    w1 = nc.dram_tensor("w1", [MPC, D, H], f32r, kind="ExternalInput").ap()
    w2 = nc.dram_tensor("w2", [MPC, H, H], f32r, kind="ExternalInput").ap()
    # host-side pre-arranged: w3[p, m, kt], b1/b2[p, m, mt]
    w3 = nc.dram_tensor("w3", [128, MPC, KT], f32r, kind="ExternalInput").ap()
    b1 = nc.dram_tensor("b1", [128, MPC, MT], f32, kind="ExternalInput").ap()
    b2 = nc.dram_tensor("b2", [128, MPC, MT], f32, kind="ExternalInput").ap()
    out = nc.dram_tensor("out", [MPC, B], f32, kind="ExternalOutput").ap()

    w1s = [nc.alloc_sbuf_tensor(f"w1s{m}", [128, KT, H], f32r).ap()
           for m in range(MPC)]
    w2s = [nc.alloc_sbuf_tensor(f"w2s{m}", [128, KT, H], f32r).ap()
           for m in range(MPC)]
    w3s = nc.alloc_sbuf_tensor("w3s", [128, MPC, KT], f32r).ap()
    b1s = nc.alloc_sbuf_tensor("b1s", [128, MPC, MT], f32).ap()
    b2s = nc.alloc_sbuf_tensor("b2s", [128, MPC, MT], f32).ap()
    xs = nc.alloc_sbuf_tensor("xs", [128, XBUF, KT, CH], f32r).ap()
    h1 = nc.alloc_sbuf_tensor("h1", [128, MPC, KT, CH], f32r).ap()
    h2 = nc.alloc_sbuf_tensor("h2", [128, MPC, KT, CH], f32r).ap()
    # per-member output staging, both at partition 0
    osb = [nc.alloc_sbuf_tensor(f"osb{m}", [1, NCH, CH], f32).ap()
           for m in range(MPC)]

    psA = nc.alloc_psum_tensor("psA", [128, 2 * MPC, CH], f32).ap()  # L1
    psB = nc.alloc_psum_tensor("psB", [128, 2 * MPC, CH], f32).ap()  # L2+L3

    # PE warmup scratch: dummy matmuls during the DMA prologue keep the HAM
    # clock-gate ramp off the critical path
    scr = nc.alloc_sbuf_tensor("scr", [128, 128 + CH], f32r).ap()
    N_WARM = _CACHE.get("n_warm_override", 28)

    xT_r = xT.rearrange("(kt p) b -> p kt b", p=128)

    # --- semaphore tick bookkeeping (absolute counts; 18 groups/chunk) ---
    # PE group order per chunk: L1(m0)x4, L1(m1)x4, L2(m0)x4, L2(m1)x4,
    # L3(m0), L3(m1). ACT mirrors it.
    def mm_l1(c, m, mt):
        return 18 * c + 4 * m + mt + 1

    def mm_l2(c, m, mt):
        return 18 * c + 8 + 4 * m + mt + 1

    def mm_l3(c, m):
        return 18 * c + 16 + m + 1

    # ACT does 16 relus per chunk (copies live on DVE)
    def act_r1(c, m, mt):
        return 16 * c + 4 * m + mt + 1

    def act_r2(c, m, mt):
        return 16 * c + 8 + 4 * m + mt + 1

    with (
        nc.Block() as block,
        nc.semaphore("mm_sem") as mm_sem,
        nc.semaphore("act_sem") as act_sem,
        nc.semaphore("b1_sem") as b1_sem,
        nc.semaphore("b2_sem") as b2_sem,
        nc.semaphore("w3_sem") as w3_sem,
        nc.semaphore("d_sem") as d_sem,
    ):
        # per-slot x semaphores: DMA queue completions are unordered across
        # chunks, so a single cumulative counter would be racy
        x_sems = [nc.alloc_semaphore(f"x_sem{s}") for s in range(XBUF)]
        cp_sem = nc.alloc_semaphore("cp_sem")
        w1_sems = [nc.alloc_semaphore(f"w1_sem{m}") for m in range(MPC)]
        w2_sems = [nc.alloc_semaphore(f"w2_sem{m}") for m in range(MPC)]

        def dma_x(sync, c):
            for kt in range(KT):
                sync.dma_start(
                    out=xs[:, c % XBUF, kt, :],
                    in_=xT_r[:, kt, c * CH:(c + 1) * CH],
                ).then_inc(x_sems[c % XBUF], 16)

        @block.sync
        def _(sync: bass.BassEngine):
            # interleave weight loads with early x chunks, ordered by need
            w1r = [w1[m].rearrange("(kt p) m2 -> p kt m2", p=128)
                   for m in range(MPC)]
            w2r = [w2[m].rearrange("(kt p) m2 -> p kt m2", p=128)
                   for m in range(MPC)]

            for kt in range(KT):
                sync.dma_start(out=w1s[0][:, kt], in_=w1r[0][:, kt]
                               ).then_inc(w1_sems[0], 16)
            sync.dma_start(out=b1s, in_=b1).then_inc(b1_sem, 16)
            dma_x(sync, 0)
            for kt in range(KT):
                sync.dma_start(out=w1s[1][:, kt], in_=w1r[1][:, kt]
                               ).then_inc(w1_sems[1], 16)
            sync.dma_start(out=b2s, in_=b2).then_inc(b2_sem, 16)
            sync.dma_start(out=w3s, in_=w3).then_inc(w3_sem, 16)
            dma_x(sync, 1)
            for kt in range(KT):
                sync.dma_start(out=w2s[0][:, kt], in_=w2r[0][:, kt]
                               ).then_inc(w2_sems[0], 16)
            dma_x(sync, 2)
            for kt in range(KT):
                sync.dma_start(out=w2s[1][:, kt], in_=w2r[1][:, kt]
                               ).then_inc(w2_sems[1], 16)
            dma_x(sync, 3)

            out_r = out.rearrange("m (nch ch) -> m nch ch", ch=CH)
            for c in range(XBUF, NCH):
                # x slot free once L1 of chunk c-XBUF fully consumed it
                sync.wait_ge(mm_sem, mm_l1(c - XBUF, MPC - 1, MT - 1))
                dma_x(sync, c)
                # trailing store for chunk c-XBUF (copies long done by now)
                cs = c - XBUF
                sync.wait_ge(cp_sem, MPC * (cs + 1))
                for m in range(MPC):
                    sync.dma_start(out=out_r[m:m + 1, cs],
                                   in_=osb[m][:, cs]).then_inc(d_sem, 16)

            for cs in range(NCH - XBUF, NCH):
                sync.wait_ge(cp_sem, MPC * (cs + 1))
                for m in range(MPC):
                    sync.dma_start(out=out_r[m:m + 1, cs],
                                   in_=osb[m][:, cs]).then_inc(d_sem, 16)
            sync.wait_ge(d_sem, 16 * MPC * NCH)

        @block.vector
        def _(vector: bass.BassEngine):
            # L3 psum -> osb copies live on DVE (otherwise idle): keeps ACT's
            # activation-table pinned to Relu
            for c in range(NCH):
                for m in range(MPC):
                    vector.wait_ge(mm_sem, mm_l3(c, m))
                    vector.tensor_copy(
                        osb[m][0:1, c, :], psB[0:1, 2 * m, :],
                    ).then_inc(cp_sem, 1)

        @block.tensor
        def _(tensor: bass.BassEngine):
            # warmup on uninitialized scratch: values are irrelevant, the psum
            # is overwritten (start=True) before any reader
            for i in range(N_WARM):
                tensor.matmul(psA[:, 0, :], scr[:, :128], scr[:, 128:],
                              start=True, stop=True, skip_group_check=True)
            for c in range(NCH):
                tensor.wait_ge(x_sems[c % XBUF], 64 * (c // XBUF + 1))
                # L1 both members
                for m in range(MPC):
                    if c == 0:
                        tensor.wait_ge(w1_sems[m], 64)
                    for mt in range(MT):
                        if mt >= 2:           # 2-bank rotation WAR
                            tensor.wait_ge(act_sem, act_r1(c, m, mt - 2))
                        elif c > 0:           # bank last used by c-1, mt+2
                            tensor.wait_ge(act_sem, act_r1(c - 1, m, mt + 2))
                        for kt in range(KT):
                            ins = tensor.matmul(
                                psA[:, 2 * m + mt % 2, :],
                                w1s[m][:, kt, mt * 128:(mt + 1) * 128],
                                xs[:, c % XBUF, kt, :],
                                start=(kt == 0), stop=(kt == KT - 1),
                            )
                        ins.then_inc(mm_sem, 1)
                # L2 both members
                for m in range(MPC):
                    if c == 0:
                        tensor.wait_ge(w2_sems[m], 64)
                    if c > 0:
                        # psB bank 2m holds chunk c-1's L3 row until DVE
                        # copies it out
                        tensor.wait_ge(cp_sem, 2 * (c - 1) + m + 1)
                    tensor.wait_ge(act_sem, act_r1(c, m, MT - 1))  # h1 ready
                    for mt in range(MT):
                        if mt >= 2:
                            tensor.wait_ge(act_sem, act_r2(c, m, mt - 2))
                        for kt in range(KT):
                            ins = tensor.matmul(
                                psB[:, 2 * m + mt % 2, :],
                                w2s[m][:, kt, mt * 128:(mt + 1) * 128],
                                h1[:, m, kt, :],
                                start=(kt == 0), stop=(kt == KT - 1),
                            )
                        ins.then_inc(mm_sem, 1)
                # L3 both members -> psB bank 2m, partition 0
                for m in range(MPC):
                    if c == 0 and m == 0:
                        tensor.wait_ge(w3_sem, 16)
                    tensor.wait_ge(act_sem, act_r2(c, m, MT - 1))  # h2 ready
                    for kt in range(KT):
                        ins = tensor.matmul(
                            psB[0:1, 2 * m, :],
                            w3s[:, m, kt:kt + 1],
                            h2[:, m, kt, :],
                            start=(kt == 0), stop=(kt == KT - 1),
                        )
                    ins.then_inc(mm_sem, 1)

        @block.scalar
        def _(scalar: bass.BassEngine):
            Relu = bass.mybir.ActivationFunctionType.Relu
            scalar.wait_ge(b1_sem, 16)
            scalar.wait_ge(b2_sem, 16)
            for c in range(NCH):
                for m in range(MPC):
                    for mt in range(MT):
                        scalar.wait_ge(mm_sem, mm_l1(c, m, mt))
                        scalar.activation(
                            h1[:, m, mt, :], psA[:, 2 * m + mt % 2, :], Relu,
                            bias=b1s[:, m, mt:mt + 1],
                        ).then_inc(act_sem, 1)
                for m in range(MPC):
                    for mt in range(MT):
                        scalar.wait_ge(mm_sem, mm_l2(c, m, mt))
                        scalar.activation(
                            h2[:, m, mt, :], psB[:, 2 * m + mt % 2, :], Relu,
                            bias=b2s[:, m, mt:mt + 1],
                        ).then_inc(act_sem, 1)

    return nc


def get_nc():
    if "nc" not in _CACHE:
        _CACHE["nc"] = _build()
    return _CACHE["nc"]


def kernel(x, W1, b1, W2, b2, W3, b3):
    from concourse.bass_utils import run_bass_kernel_spmd

    nc = get_nc()
    xT = np.ascontiguousarray(np.asarray(x, dtype=np.float32).T)
    W1 = np.asarray(W1, dtype=np.float32)
    W2 = np.asarray(W2, dtype=np.float32)
    W3 = np.asarray(W3, dtype=np.float32)
    b1 = np.asarray(b1, dtype=np.float32)
    b2 = np.asarray(b2, dtype=np.float32)
    b3 = np.asarray(b3, dtype=np.float32)

    def feat_major(v):
        # [MPC, H] -> [128, MPC, H//128]: v[p, m, t] = v_in[m, t*128 + p]
        return np.ascontiguousarray(
            v.reshape(MPC, H // 128, 128).transpose(2, 0, 1))

    in_maps = []
    for c in range(N_CORES):
        s = slice(MPC * c, MPC * (c + 1))
        in_maps.append({
            "xT": xT,
            "w1": np.ascontiguousarray(W1[s]),
            "w2": np.ascontiguousarray(W2[s]),
            "w3": feat_major(W3[s, :, 0]),
            "b1": feat_major(b1[s]),
            "b2": feat_major(b2[s]),
        })

    res = run_bass_kernel_spmd(nc, in_maps, list(range(N_CORES)))
    out = np.concatenate([r["out"] for r in res.results], axis=0)  # [E, B]
    out = out + b3.reshape(E, 1)
    return out.reshape(E, B, 1).astype(np.float32)


# revision 39
# speedup vs baseline: 1.0110x; 1.0110x over previous
"""EnsembleFC (E=16 MLPs, 512->512->512->1, relu) on 8 TRN2 NeuronCores.

Strategy (expert parallel): each core owns E/8 = 2 ensemble members' weights
and computes their [B] output column; x is replicated. All activations stay
in "feature-major" (transposed) layout so no on-device transposes are needed:

    h1^T = relu(W1^T @ x^T + b1)      [H, B]
    h2^T = relu(W2^T @ h1^T + b2)     [H, B]
    out^T = W3^T @ h2^T               [1, B]   (b3 added on host)

Matmuls run in float32r (TRN2 reduced-precision fp32 PE mode, 1 cycle/row --
4x faster than plain fp32, ~20x more accurate than bf16; measured scaled
error ~1.5e-4 per 128-deep contraction with raw fp32 inputs).

Raw Bass (no Tile framework): this container's walrus rejects instructions
with more than a couple of sync waits, which Tile's auto-generated drains
exceed. Explicit per-engine programs with standalone waits keep every
instruction at one wait.

Pipeline per chunk of 512 batch columns (PE order, software-pipelined):
  ... L1(c,m0) L1(c,m1) L3(c-1,m0) L3(c-1,m1) L2(c,m0) L2(c,m1) ...
  PSUM: each member-layer pair owns 2 banks (mt % 2 rotation); L3 reuses the
       member's first L2 bank at partition 0.
  ACT: relu+bias drains psum into h1/h2 (f32r).
  DVE: reduces h2 over k-tiles with the w3 weights in exact fp32
       (t_r = sum_kt w3[kt] * h2[kt], rounded to f32r at the end), so L3 is a
       single ones-vector matmul per member-chunk instead of four; also
       copies L3 psum rows to the output staging buffer.
  SP:  weight DMAs (per-tensor sems, split per k-tile), x chunk DMAs
       (per-slot sems -- DMA queue completions are unordered), output stores.
A short burst of dummy matmuls on scratch SBUF during the DMA prologue keeps
the PE HAM clock-gate warm so chunk 0 runs at full clock.
"""
import numpy as np

E, D, H, B = 16, 512, 512, 8192
N_CORES = 8
MPC = E // N_CORES          # members per core
KT = D // 128               # k-tiles per 512 contraction
MT = H // 128               # m-tiles per 512 output dim
CH = 512                    # batch columns per chunk (one psum bank)
NCH = B // CH               # chunks
XBUF = 4                    # x chunk buffering

_CACHE = {}


def _build():
    import concourse.bass as bass
    from concourse import mybir

    f32 = mybir.dt.float32
    f32r = mybir.dt.float32r

    nc = bass.Bass("TRN2", target_bir_lowering=False, debug=False,
                   num_devices=N_CORES)

    xT = nc.dram_tensor("xT", [D, B], f32r, kind="ExternalInput").ap()
    w1 = nc.dram_tensor("w1", [MPC, D, H], f32r, kind="ExternalInput").ap()
    w2 = nc.dram_tensor("w2", [MPC, H, H], f32r, kind="ExternalInput").ap()
    # host-side pre-arranged: w3[p, m, kt], b1/b2[p, m, mt]
    w3 = nc.dram_tensor("w3", [128, MPC, KT], f32r, kind="ExternalInput").ap()
    b1 = nc.dram_tensor("b1", [128, MPC, MT], f32, kind="ExternalInput").ap()
    b2 = nc.dram_tensor("b2", [128, MPC, MT], f32, kind="ExternalInput").ap()
    one = nc.dram_tensor("one", [128, 1], f32r, kind="ExternalInput").ap()
    out = nc.dram_tensor("out", [MPC, B], f32, kind="ExternalOutput").ap()

    w1s = [nc.alloc_sbuf_tensor(f"w1s{m}", [128, KT, H], f32r).ap()
           for m in range(MPC)]
    w2s = [nc.alloc_sbuf_tensor(f"w2s{m}", [128, KT, H], f32r).ap()
           for m in range(MPC)]
    w3s = nc.alloc_sbuf_tensor("w3s", [128, MPC, KT], f32r).ap()
    b1s = nc.alloc_sbuf_tensor("b1s", [128, MPC, MT], f32).ap()
    b2s = nc.alloc_sbuf_tensor("b2s", [128, MPC, MT], f32).ap()
    ones_s = nc.alloc_sbuf_tensor("ones_s", [128, 1], f32r).ap()
    xs = nc.alloc_sbuf_tensor("xs", [128, XBUF, KT, CH], f32r).ap()
    h1 = nc.alloc_sbuf_tensor("h1", [128, MPC, KT, CH], f32r).ap()
    h2 = nc.alloc_sbuf_tensor("h2", [128, MPC, KT, CH], f32r).ap()
    # DVE kt-reduction scratch (no aliasing: A,B pair-products, C,D partials)
    rA = nc.alloc_sbuf_tensor("rA", [128, CH], f32).ap()
    rB = nc.alloc_sbuf_tensor("rB", [128, CH], f32).ap()
    rC = nc.alloc_sbuf_tensor("rC", [128, CH], f32).ap()
    rD = nc.alloc_sbuf_tensor("rD", [128, CH], f32).ap()
    t_r = nc.alloc_sbuf_tensor("t_r", [128, MPC, CH], f32r).ap()
    # per-member output staging, both at partition 0
    osb = [nc.alloc_sbuf_tensor(f"osb{m}", [1, NCH, CH], f32).ap()
           for m in range(MPC)]

    psA = nc.alloc_psum_tensor("psA", [128, 2 * MPC, CH], f32).ap()  # L1
    psB = nc.alloc_psum_tensor("psB", [128, 2 * MPC, CH], f32).ap()  # L2+L3

    # PE warmup scratch: dummy matmuls during the DMA prologue keep the HAM
    # clock-gate ramp off the critical path (uninitialized on HW -- harmless)
    scr = nc.alloc_sbuf_tensor("scr", [128, 128 + CH], f32r).ap()
    N_WARM = _CACHE.get("n_warm_override", 28)

    xT_r = xT.rearrange("(kt p) b -> p kt b", p=128)

    # --- tick tables (absolute semaphore counts, mirror emission order) ---
    mmT = {}
    _t = 0
    for c in range(NCH):
        for m in range(MPC):
            for mt in range(MT):
                _t += 1
                mmT[("l1", c, m, mt)] = _t
        if c >= 1:
            for m in range(MPC):
                _t += 1
                mmT[("l3", c - 1, m)] = _t
        for m in range(MPC):
            for mt in range(MT):
                _t += 1
                mmT[("l2", c, m, mt)] = _t
    for m in range(MPC):
        _t += 1
        mmT[("l3", NCH - 1, m)] = _t

    def act_r1(c, m, mt):
        return 16 * c + 4 * m + mt + 1

    def act_r2(c, m, mt):
        return 16 * c + 8 + 4 * m + mt + 1

    def dve_red(c, m):
        return 4 * c + m + 1

    def dve_cp(c, m):
        return 4 * c + 2 + m + 1

    with (
        nc.Block() as block,
        nc.semaphore("mm_sem") as mm_sem,
        nc.semaphore("act_sem") as act_sem,
        nc.semaphore("b1_sem") as b1_sem,
        nc.semaphore("b2_sem") as b2_sem,
        nc.semaphore("w3_sem") as w3_sem,
        nc.semaphore("d_sem") as d_sem,
    ):
        # per-slot x semaphores: DMA queue completions are unordered across
        # chunks, so a single cumulative counter would be racy
        x_sems = [nc.alloc_semaphore(f"x_sem{s}") for s in range(XBUF)]
        dve_sem = nc.alloc_semaphore("dve_sem")
        rd_sem = nc.alloc_semaphore("rd_sem")   # intra-DVE RAW/WAR ordering
        w1_sems = [nc.alloc_semaphore(f"w1_sem{m}") for m in range(MPC)]
        w2_sems = [nc.alloc_semaphore(f"w2_sem{m}") for m in range(MPC)]

        def dma_x(sync, c):
            for kt in range(KT):
                sync.dma_start(
                    out=xs[:, c % XBUF, kt, :],
                    in_=xT_r[:, kt, c * CH:(c + 1) * CH],
                ).then_inc(x_sems[c % XBUF], 16)

        @block.sync
        def _(sync: bass.BassEngine):
            # interleave weight loads with early x chunks, ordered by need
            w1r = [w1[m].rearrange("(kt p) m2 -> p kt m2", p=128)
                   for m in range(MPC)]
            w2r = [w2[m].rearrange("(kt p) m2 -> p kt m2", p=128)
                   for m in range(MPC)]
            for kt in range(KT):
                sync.dma_start(out=w1s[0][:, kt], in_=w1r[0][:, kt]
                               ).then_inc(w1_sems[0], 16)
            sync.dma_start(out=b1s, in_=b1).then_inc(b1_sem, 16)
            dma_x(sync, 0)
            for kt in range(KT):
                sync.dma_start(out=w1s[1][:, kt], in_=w1r[1][:, kt]
                               ).then_inc(w1_sems[1], 16)
            sync.dma_start(out=b2s, in_=b2).then_inc(b2_sem, 16)
            sync.dma_start(out=w3s, in_=w3).then_inc(w3_sem, 16)
            sync.dma_start(out=ones_s, in_=one).then_inc(w3_sem, 16)
            dma_x(sync, 1)
            for kt in range(KT):
                sync.dma_start(out=w2s[0][:, kt], in_=w2r[0][:, kt]
                               ).then_inc(w2_sems[0], 16)
            dma_x(sync, 2)
            for kt in range(KT):
                sync.dma_start(out=w2s[1][:, kt], in_=w2r[1][:, kt]
                               ).then_inc(w2_sems[1], 16)
            dma_x(sync, 3)

            out_r = out.rearrange("m (nch ch) -> m nch ch", ch=CH)
            for c in range(XBUF, NCH):
                # x slot free once L1 of chunk c-XBUF fully consumed it
                sync.wait_ge(mm_sem, mmT[("l1", c - XBUF, MPC - 1, MT - 1)])
                dma_x(sync, c)
                # trailing store for chunk c-XBUF
                cs = c - XBUF
                sync.wait_ge(dve_sem, dve_cp(cs, MPC - 1))
                for m in range(MPC):
                    sync.dma_start(out=out_r[m:m + 1, cs],
                                   in_=osb[m][:, cs]).then_inc(d_sem, 16)

            for cs in range(NCH - XBUF, NCH):
                sync.wait_ge(dve_sem, dve_cp(cs, MPC - 1))
                for m in range(MPC):
                    sync.dma_start(out=out_r[m:m + 1, cs],
                                   in_=osb[m][:, cs]).then_inc(d_sem, 16)
            sync.wait_ge(d_sem, 16 * MPC * NCH)

        @block.vector
        def _(vector: bass.BassEngine):
            # DVE: (a) kt-reduction t_r = sum_kt w3[kt]*h2[kt] in exact fp32
            # (takes 3 of every 4 L3 matmuls off the PE, and is more accurate
            # than f32r products), (b) L3 psum -> osb copies.
            w3f = w3s.bitcast(f32)
            for c in range(NCH):
                for m in range(MPC):
                    # h2 ready; implies PE already read t_r(c-1, m) (its L3
                    # precedes this chunk's L2 in the PE stream)
                    vector.wait_ge(act_sem, act_r2(c, m, MT - 1))
                    h2f = h2[:, m].bitcast(f32)
                    # DVE does not self-interlock same-engine RAW/WAR;
                    # rd_sem orders the reduction chain explicitly
                    base = 6 * (MPC * c + m)
                    if base:
                        vector.wait_ge(rd_sem, base)
                    vector.tensor_scalar_mul(rA, h2f[:, 0, :], w3f[:, m, 0:1]
                                             ).then_inc(rd_sem, 1)
                    vector.tensor_scalar_mul(rB, h2f[:, 1, :], w3f[:, m, 1:2]
                                             ).then_inc(rd_sem, 1)
                    vector.wait_ge(rd_sem, base + 2)
                    vector.tensor_add(rC, rA, rB).then_inc(rd_sem, 1)
                    vector.wait_ge(rd_sem, base + 3)   # addC done before rA/rB reuse
                    vector.tensor_scalar_mul(rA, h2f[:, 2, :], w3f[:, m, 2:3]
                                             ).then_inc(rd_sem, 1)
                    vector.tensor_scalar_mul(rB, h2f[:, 3, :], w3f[:, m, 3:4]
                                             ).then_inc(rd_sem, 1)
                    vector.wait_ge(rd_sem, base + 5)
                    vector.tensor_add(rD, rA, rB).then_inc(rd_sem, 1)
                    vector.wait_ge(rd_sem, base + 6)
                    vector.tensor_add(t_r[:, m, :], rC, rD
                                      ).then_inc(dve_sem, 1)
                for m in range(MPC):
                    vector.wait_ge(mm_sem, mmT[("l3", c, m)])
                    vector.tensor_copy(
                        osb[m][0:1, c, :], psB[0:1, 2 * m, :],
                    ).then_inc(dve_sem, 1)

        @block.tensor
        def _(tensor: bass.BassEngine):
            # warmup on uninitialized scratch: values are irrelevant, the psum
            # is overwritten (start=True) before any reader
            for i in range(N_WARM):
                tensor.matmul(psA[:, 0, :], scr[:, :128], scr[:, 128:],
                              start=True, stop=True, skip_group_check=True)

            def l3(c, m):
                # single ones-matmul over the DVE-reduced t_r
                tensor.wait_ge(dve_sem, dve_red(c, m))
                tensor.matmul(
                    psB[0:1, 2 * m, :], ones_s, t_r[:, m, :],
                    start=True, stop=True,
                ).then_inc(mm_sem, 1)

            for c in range(NCH):
                tensor.wait_ge(x_sems[c % XBUF], 64 * (c // XBUF + 1))
                # L1 both members
                for m in range(MPC):
                    if c == 0:
                        tensor.wait_ge(w1_sems[m], 64)
                    for mt in range(MT):
                        if mt >= 2:           # 2-bank rotation WAR
                            tensor.wait_ge(act_sem, act_r1(c, m, mt - 2))
                        elif c > 0:           # bank last used by c-1, mt+2
                            tensor.wait_ge(act_sem, act_r1(c - 1, m, mt + 2))
                        for kt in range(KT):
                            ins = tensor.matmul(
                                psA[:, 2 * m + mt % 2, :],
                                w1s[m][:, kt, mt * 128:(mt + 1) * 128],
                                xs[:, c % XBUF, kt, :],
                                start=(kt == 0), stop=(kt == KT - 1),
                            )
                        ins.then_inc(mm_sem, 1)
                # pipelined L3 of the previous chunk: its DVE reduction ran
                # while this chunk's L1 was on the PE
                if c >= 1:
                    if c == 1:
                        tensor.wait_ge(w3_sem, 32)
                    for m in range(MPC):
                        l3(c - 1, m)
                # L2 both members
                for m in range(MPC):
                    if c == 0:
                        tensor.wait_ge(w2_sems[m], 64)
                    if c > 0:
                        # psB bank 2m holds chunk c-1's L3 row until DVE
                        # copies it out
                        tensor.wait_ge(dve_sem, dve_cp(c - 1, m))
                    tensor.wait_ge(act_sem, act_r1(c, m, MT - 1))  # h1 ready
                    for mt in range(MT):
                        if mt >= 2:
                            tensor.wait_ge(act_sem, act_r2(c, m, mt - 2))
                        for kt in range(KT):
                            ins = tensor.matmul(
                                psB[:, 2 * m + mt % 2, :],
                                w2s[m][:, kt, mt * 128:(mt + 1) * 128],
                                h1[:, m, kt, :],
                                start=(kt == 0), stop=(kt == KT - 1),
                            )
                        ins.then_inc(mm_sem, 1)
            for m in range(MPC):
                l3(NCH - 1, m)

        @block.scalar
        def _(scalar: bass.BassEngine):
            Relu = bass.mybir.ActivationFunctionType.Relu
            scalar.wait_ge(b1_sem, 16)
            scalar.wait_ge(b2_sem, 16)
            for c in range(NCH):
                for m in range(MPC):
                    for mt in range(MT):
                        scalar.wait_ge(mm_sem, mmT[("l1", c, m, mt)])
                        scalar.activation(
                            h1[:, m, mt, :], psA[:, 2 * m + mt % 2, :], Relu,
                            bias=b1s[:, m, mt:mt + 1],
                        ).then_inc(act_sem, 1)
                for m in range(MPC):
                    for mt in range(MT):
                        scalar.wait_ge(mm_sem, mmT[("l2", c, m, mt)])
                        scalar.activation(
                            h2[:, m, mt, :], psB[:, 2 * m + mt % 2, :], Relu,
                            bias=b2s[:, m, mt:mt + 1],
                        ).then_inc(act_sem, 1)

    return nc


def get_nc():
    if "nc" not in _CACHE:
        _CACHE["nc"] = _build()
    return _CACHE["nc"]


def kernel(x, W1, b1, W2, b2, W3, b3):
    from concourse.bass_utils import run_bass_kernel_spmd

    nc = get_nc()
    xT = np.ascontiguousarray(np.asarray(x, dtype=np.float32).T)
    W1 = np.asarray(W1, dtype=np.float32)
    W2 = np.asarray(W2, dtype=np.float32)
    W3 = np.asarray(W3, dtype=np.float32)
    b1 = np.asarray(b1, dtype=np.float32)
    b2 = np.asarray(b2, dtype=np.float32)
    b3 = np.asarray(b3, dtype=np.float32)

    def feat_major(v):
        # [MPC, H] -> [128, MPC, H//128]: v[p, m, t] = v_in[m, t*128 + p]
        return np.ascontiguousarray(
            v.reshape(MPC, H // 128, 128).transpose(2, 0, 1))

    in_maps = []
    for c in range(N_CORES):
        s = slice(MPC * c, MPC * (c + 1))
        in_maps.append({
            "xT": xT,
            "w1": np.ascontiguousarray(W1[s]),
            "w2": np.ascontiguousarray(W2[s]),
            "w3": feat_major(W3[s, :, 0]),
            "b1": feat_major(b1[s]),
            "b2": feat_major(b2[s]),
            "one": np.ones((128, 1), dtype=np.float32),
        })

    res = run_bass_kernel_spmd(nc, in_maps, list(range(N_CORES)))
    out = np.concatenate([r["out"] for r in res.results], axis=0)  # [E, B]
    out = out + b3.reshape(E, 1)
    return out.reshape(E, B, 1).astype(np.float32)


# revision 40
# speedup vs baseline: 1.0622x; 1.0506x over previous
"""EnsembleFC (E=16 MLPs, 512->512->512->1, relu) on 8 TRN2 NeuronCores.

Strategy (expert parallel): each core owns E/8 = 2 ensemble members' weights
and computes their [B] output column; x is replicated. All activations stay
in "feature-major" (transposed) layout so no on-device transposes are needed:

    h1^T = relu(W1^T @ x^T + b1)      [H, B]
    h2^T = relu(W2^T @ h1^T + b2)     [H, B]
    out^T = W3^T @ h2^T               [1, B]   (b3 added on host)

Matmuls run in float32r (TRN2 reduced-precision fp32 PE mode, 1 cycle/row --
4x faster than plain fp32, ~20x more accurate than bf16; measured scaled
error ~1.5e-4 per 128-deep contraction with raw fp32 inputs).

Raw Bass (no Tile framework): this container's walrus rejects instructions
with more than a couple of sync waits, which Tile's auto-generated drains
exceed. Explicit per-engine programs with standalone waits keep every
instruction at one wait.

Pipeline per chunk of 512 batch columns (PE order, software-pipelined):
  ... L1(c,m0) L1(c,m1) L3(c-1,m0) L3(c-1,m1) L2(c,m0) L2(c,m1) ...
  PSUM: each member-layer pair owns 2 banks (mt % 2 rotation); L3 reuses the
       member's first L2 bank at partition 0.
  ACT: relu+bias drains psum into h1/h2 (f32r).
  DVE: reduces h2 over k-tiles with the w3 weights in exact fp32
       (t_r = sum_kt w3[kt] * h2[kt], rounded to f32r at the end), so L3 is a
       single ones-vector matmul per member-chunk instead of four; also
       copies L3 psum rows to the output staging buffer.
  SP:  weight DMAs (per-tensor sems, split per k-tile), x chunk DMAs
       (per-slot sems -- DMA queue completions are unordered), output stores.
A short burst of dummy matmuls on scratch SBUF during the DMA prologue keeps
the PE HAM clock-gate warm so chunk 0 runs at full clock.
"""
import numpy as np

E, D, H, B = 16, 512, 512, 8192
N_CORES = 8
MPC = E // N_CORES          # members per core
KT = D // 128               # k-tiles per 512 contraction
MT = H // 128               # m-tiles per 512 output dim
CH = 512                    # batch columns per chunk (one psum bank)
NCH = B // CH               # chunks
XBUF = 4                    # x chunk buffering

_CACHE = {}


def _build():
    import concourse.bass as bass
    from concourse import mybir

    f32 = mybir.dt.float32
    f32r = mybir.dt.float32r

    nc = bass.Bass("TRN2", target_bir_lowering=False, debug=False,
                   num_devices=N_CORES)

    xT = nc.dram_tensor("xT", [D, B], f32r, kind="ExternalInput").ap()
    w1 = nc.dram_tensor("w1", [MPC, D, H], f32r, kind="ExternalInput").ap()
    w2 = nc.dram_tensor("w2", [MPC, H, H], f32r, kind="ExternalInput").ap()
    # host-side pre-arranged: w3[p, m, kt], b1/b2[p, m, mt]
    w3 = nc.dram_tensor("w3", [128, MPC, KT], f32r, kind="ExternalInput").ap()
    b1 = nc.dram_tensor("b1", [128, MPC, MT], f32, kind="ExternalInput").ap()
    b2 = nc.dram_tensor("b2", [128, MPC, MT], f32, kind="ExternalInput").ap()
    one = nc.dram_tensor("one", [128, 1], f32r, kind="ExternalInput").ap()
    out = nc.dram_tensor("out", [MPC, B], f32, kind="ExternalOutput").ap()

    w1s = [nc.alloc_sbuf_tensor(f"w1s{m}", [128, KT, H], f32r).ap()
           for m in range(MPC)]
    w2s = [nc.alloc_sbuf_tensor(f"w2s{m}", [128, KT, H], f32r).ap()
           for m in range(MPC)]
    w3s = nc.alloc_sbuf_tensor("w3s", [128, MPC, KT], f32r).ap()
    b1s = nc.alloc_sbuf_tensor("b1s", [128, MPC, MT], f32).ap()
    b2s = nc.alloc_sbuf_tensor("b2s", [128, MPC, MT], f32).ap()
    ones_s = nc.alloc_sbuf_tensor("ones_s", [128, 1], f32r).ap()
    xs = nc.alloc_sbuf_tensor("xs", [128, XBUF, KT, CH], f32r).ap()
    h1 = nc.alloc_sbuf_tensor("h1", [128, MPC, KT, CH], f32r).ap()
    h2 = nc.alloc_sbuf_tensor("h2", [128, MPC, KT, CH], f32r).ap()
    # DVE kt-reduction scratch (no aliasing: A,B pair-products, C,D partials)
    rA = nc.alloc_sbuf_tensor("rA", [128, CH], f32).ap()
    rB = nc.alloc_sbuf_tensor("rB", [128, CH], f32).ap()
    rC = nc.alloc_sbuf_tensor("rC", [128, CH], f32).ap()
    rD = nc.alloc_sbuf_tensor("rD", [128, CH], f32).ap()
    t_r = nc.alloc_sbuf_tensor("t_r", [128, MPC, CH], f32r).ap()
    # per-member output staging, both at partition 0
    osb = [nc.alloc_sbuf_tensor(f"osb{m}", [1, NCH, CH], f32).ap()
           for m in range(MPC)]

    psA = nc.alloc_psum_tensor("psA", [128, 2 * MPC, CH], f32).ap()  # L1
    psB = nc.alloc_psum_tensor("psB", [128, 2 * MPC, CH], f32).ap()  # L2+L3

    # PE warmup scratch: dummy matmuls during the DMA prologue keep the HAM
    # clock-gate ramp off the critical path (uninitialized on HW -- harmless)
    scr = nc.alloc_sbuf_tensor("scr", [128, 128 + CH], f32r).ap()
    N_WARM = _CACHE.get("n_warm_override", 28)

    xT_r = xT.rearrange("(kt p) b -> p kt b", p=128)

    # --- tick tables (absolute semaphore counts, mirror emission order) ---
    mmT = {}
    _t = 0
    for c in range(NCH):
        for m in range(MPC):
            for mt in range(MT):
                _t += 1
                mmT[("l1", c, m, mt)] = _t
        if c >= 1:
            for m in range(MPC):
                _t += 1
                mmT[("l3", c - 1, m)] = _t
        for m in range(MPC):
            for mt in range(MT):
                _t += 1
                mmT[("l2", c, m, mt)] = _t
    for m in range(MPC):
        _t += 1
        mmT[("l3", NCH - 1, m)] = _t

    def act_r1(c, m, mt):
        return 16 * c + 4 * m + mt + 1

    def act_r2(c, m, mt):
        return 16 * c + 8 + 4 * m + mt + 1

    def dve_red(c, m):
        return 4 * c + m + 1

    def dve_cp(c, m):
        return 4 * c + 2 + m + 1

    with (
        nc.Block() as block,
        nc.semaphore("mm_sem") as mm_sem,
        nc.semaphore("act_sem") as act_sem,
        nc.semaphore("b1_sem") as b1_sem,
        nc.semaphore("b2_sem") as b2_sem,
        nc.semaphore("w3_sem") as w3_sem,
        nc.semaphore("d_sem") as d_sem,
    ):
        # per-slot x semaphores: DMA queue completions are unordered across
        # chunks, so a single cumulative counter would be racy
        x_sems = [nc.alloc_semaphore(f"x_sem{s}") for s in range(XBUF)]
        dve_sem = nc.alloc_semaphore("dve_sem")
        rd_sem = nc.alloc_semaphore("rd_sem")   # intra-DVE RAW/WAR ordering
        w1_sems = [nc.alloc_semaphore(f"w1_sem{m}") for m in range(MPC)]
        w2_sems = [nc.alloc_semaphore(f"w2_sem{m}") for m in range(MPC)]

        def dma_x(sync, c):
            for kt in range(KT):
                sync.dma_start(
                    out=xs[:, c % XBUF, kt, :],
                    in_=xT_r[:, kt, c * CH:(c + 1) * CH],
                ).then_inc(x_sems[c % XBUF], 16)

        @block.sync
        def _(sync: bass.BassEngine):
            # interleave weight loads with early x chunks, ordered by need
            w1r = [w1[m].rearrange("(kt p) m2 -> p kt m2", p=128)
                   for m in range(MPC)]
            w2r = [w2[m].rearrange("(kt p) m2 -> p kt m2", p=128)
                   for m in range(MPC)]
            for kt in range(KT):
                sync.dma_start(out=w1s[0][:, kt], in_=w1r[0][:, kt]
                               ).then_inc(w1_sems[0], 16)
            sync.dma_start(out=b1s, in_=b1).then_inc(b1_sem, 16)
            dma_x(sync, 0)
            for kt in range(KT):
                sync.dma_start(out=w1s[1][:, kt], in_=w1r[1][:, kt]
                               ).then_inc(w1_sems[1], 16)
            sync.dma_start(out=b2s, in_=b2).then_inc(b2_sem, 16)
            sync.dma_start(out=w3s, in_=w3).then_inc(w3_sem, 16)
            sync.dma_start(out=ones_s, in_=one).then_inc(w3_sem, 16)
            dma_x(sync, 1)
            for kt in range(KT):
                sync.dma_start(out=w2s[0][:, kt], in_=w2r[0][:, kt]
                               ).then_inc(w2_sems[0], 16)
            dma_x(sync, 2)
            for kt in range(KT):
                sync.dma_start(out=w2s[1][:, kt], in_=w2r[1][:, kt]
                               ).then_inc(w2_sems[1], 16)
            dma_x(sync, 3)

            out_r = out.rearrange("m (nch ch) -> m nch ch", ch=CH)
            for c in range(XBUF, NCH):
                # x slot free once L1 of chunk c-XBUF fully consumed it
                sync.wait_ge(mm_sem, mmT[("l1", c - XBUF, MPC - 1, MT - 1)])
                dma_x(sync, c)
                # trailing store for chunk c-XBUF
                cs = c - XBUF
                sync.wait_ge(dve_sem, dve_cp(cs, MPC - 1))
                for m in range(MPC):
                    sync.dma_start(out=out_r[m:m + 1, cs],
                                   in_=osb[m][:, cs]).then_inc(d_sem, 16)

            for cs in range(NCH - XBUF, NCH):
                sync.wait_ge(dve_sem, dve_cp(cs, MPC - 1))
                for m in range(MPC):
                    sync.dma_start(out=out_r[m:m + 1, cs],
                                   in_=osb[m][:, cs]).then_inc(d_sem, 16)
            sync.wait_ge(d_sem, 16 * MPC * NCH)

        @block.vector
        def _(vector: bass.BassEngine):
            # DVE: (a) kt-reduction t_r = sum_kt w3[kt]*h2[kt] in exact fp32
            # (takes 3 of every 4 L3 matmuls off the PE, and is more accurate
            # than f32r products), (b) L3 psum -> osb copies.
            w3f = w3s.bitcast(f32)
            for c in range(NCH):
                for m in range(MPC):
                    # h2 ready; implies PE already read t_r(c-1, m) (its L3
                    # precedes this chunk's L2 in the PE stream)
                    vector.wait_ge(act_sem, act_r2(c, m, MT - 1))
                    h2f = h2[:, m].bitcast(f32)
                    # DVE does not self-interlock same-engine RAW/WAR;
                    # rd_sem orders the reduction chain explicitly
                    base = 6 * (MPC * c + m)
                    if base:
                        vector.wait_ge(rd_sem, base)
                    vector.tensor_scalar_mul(rA, h2f[:, 0, :], w3f[:, m, 0:1]
                                             ).then_inc(rd_sem, 1)
                    vector.tensor_scalar_mul(rB, h2f[:, 1, :], w3f[:, m, 1:2]
                                             ).then_inc(rd_sem, 1)
                    vector.wait_ge(rd_sem, base + 2)
                    vector.tensor_add(rC, rA, rB).then_inc(rd_sem, 1)
                    vector.wait_ge(rd_sem, base + 3)   # addC done before rA/rB reuse
                    vector.tensor_scalar_mul(rA, h2f[:, 2, :], w3f[:, m, 2:3]
                                             ).then_inc(rd_sem, 1)
                    vector.tensor_scalar_mul(rB, h2f[:, 3, :], w3f[:, m, 3:4]
                                             ).then_inc(rd_sem, 1)
                    vector.wait_ge(rd_sem, base + 5)
                    vector.tensor_add(rD, rA, rB).then_inc(rd_sem, 1)
                    vector.wait_ge(rd_sem, base + 6)
                    vector.tensor_add(t_r[:, m, :], rC, rD
                                      ).then_inc(dve_sem, 1)
                for m in range(MPC):
                    vector.wait_ge(mm_sem, mmT[("l3", c, m)])
                    vector.tensor_copy(
                        osb[m][0:1, c, :], psB[0:1, 2 * m + 1, :],
                    ).then_inc(dve_sem, 1)

        @block.tensor
        def _(tensor: bass.BassEngine):
            # warmup on uninitialized scratch: values are irrelevant, the psum
            # is overwritten (start=True) before any reader
            for i in range(N_WARM):
                tensor.matmul(psA[:, 0, :], scr[:, :128], scr[:, 128:],
                              start=True, stop=True, skip_group_check=True)

            def l3(c, m):
                # single ones-matmul over the DVE-reduced t_r; bank 2m+1 so
                # the osb copy only gates the SECOND L2 group of chunk c+1
                tensor.wait_ge(dve_sem, dve_red(c, m))
                tensor.matmul(
                    psB[0:1, 2 * m + 1, :], ones_s, t_r[:, m, :],
                    start=True, stop=True,
                ).then_inc(mm_sem, 1)

            for c in range(NCH):
                tensor.wait_ge(x_sems[c % XBUF], 64 * (c // XBUF + 1))
                # L1 both members
                for m in range(MPC):
                    if c == 0:
                        tensor.wait_ge(w1_sems[m], 64)
                    for mt in range(MT):
                        if mt >= 2:           # 2-bank rotation WAR
                            tensor.wait_ge(act_sem, act_r1(c, m, mt - 2))
                        elif c > 0:           # bank last used by c-1, mt+2
                            tensor.wait_ge(act_sem, act_r1(c - 1, m, mt + 2))
                        for kt in range(KT):
                            ins = tensor.matmul(
                                psA[:, 2 * m + mt % 2, :],
                                w1s[m][:, kt, mt * 128:(mt + 1) * 128],
                                xs[:, c % XBUF, kt, :],
                                start=(kt == 0), stop=(kt == KT - 1),
                            )
                        ins.then_inc(mm_sem, 1)
                # pipelined L3 of the previous chunk: its DVE reduction ran
                # while this chunk's L1 was on the PE
                if c >= 1:
                    if c == 1:
                        tensor.wait_ge(w3_sem, 32)
                    for m in range(MPC):
                        l3(c - 1, m)
                # L2 both members
                for m in range(MPC):
                    if c == 0:
                        tensor.wait_ge(w2_sems[m], 64)
                    tensor.wait_ge(act_sem, act_r1(c, m, MT - 1))  # h1 ready
                    for mt in range(MT):
                        if mt == 1 and c > 0:
                            # psB bank 2m+1 holds chunk c-1's L3 row until
                            # DVE copies it out
                            tensor.wait_ge(dve_sem, dve_cp(c - 1, m))
                        if mt >= 2:
                            tensor.wait_ge(act_sem, act_r2(c, m, mt - 2))
                        for kt in range(KT):
                            ins = tensor.matmul(
                                psB[:, 2 * m + mt % 2, :],
                                w2s[m][:, kt, mt * 128:(mt + 1) * 128],
                                h1[:, m, kt, :],
                                start=(kt == 0), stop=(kt == KT - 1),
                            )
                        ins.then_inc(mm_sem, 1)
            for m in range(MPC):
                l3(NCH - 1, m)

        @block.scalar
        def _(scalar: bass.BassEngine):
            Relu = bass.mybir.ActivationFunctionType.Relu
            scalar.wait_ge(b1_sem, 16)
            scalar.wait_ge(b2_sem, 16)
            for c in range(NCH):
                for m in range(MPC):
                    for mt in range(MT):
                        scalar.wait_ge(mm_sem, mmT[("l1", c, m, mt)])
                        scalar.activation(
                            h1[:, m, mt, :], psA[:, 2 * m + mt % 2, :], Relu,
                            bias=b1s[:, m, mt:mt + 1],
                        ).then_inc(act_sem, 1)
                for m in range(MPC):
                    for mt in range(MT):
                        scalar.wait_ge(mm_sem, mmT[("l2", c, m, mt)])
                        scalar.activation(
                            h2[:, m, mt, :], psB[:, 2 * m + mt % 2, :], Relu,
                            bias=b2s[:, m, mt:mt + 1],
                        ).then_inc(act_sem, 1)

    return nc


def get_nc():
    if "nc" not in _CACHE:
        _CACHE["nc"] = _build()
    return _CACHE["nc"]


def kernel(x, W1, b1, W2, b2, W3, b3):
    from concourse.bass_utils import run_bass_kernel_spmd

    nc = get_nc()
    xT = np.ascontiguousarray(np.asarray(x, dtype=np.float32).T)
    W1 = np.asarray(W1, dtype=np.float32)
    W2 = np.asarray(W2, dtype=np.float32)
    W3 = np.asarray(W3, dtype=np.float32)
    b1 = np.asarray(b1, dtype=np.float32)
    b2 = np.asarray(b2, dtype=np.float32)
    b3 = np.asarray(b3, dtype=np.float32)

    def feat_major(v):
        # [MPC, H] -> [128, MPC, H//128]: v[p, m, t] = v_in[m, t*128 + p]
        return np.ascontiguousarray(
            v.reshape(MPC, H // 128, 128).transpose(2, 0, 1))

    in_maps = []
    for c in range(N_CORES):
        s = slice(MPC * c, MPC * (c + 1))
        in_maps.append({
            "xT": xT,
            "w1": np.ascontiguousarray(W1[s]),
            "w2": np.ascontiguousarray(W2[s]),
            "w3": feat_major(W3[s, :, 0]),
            "b1": feat_major(b1[s]),
            "b2": feat_major(b2[s]),
            "one": np.ones((128, 1), dtype=np.float32),
        })

    res = run_bass_kernel_spmd(nc, in_maps, list(range(N_CORES)))
    out = np.concatenate([r["out"] for r in res.results], axis=0)  # [E, B]
    out = out + b3.reshape(E, 1)
    return out.reshape(E, B, 1).astype(np.float32)


# revision 41
# speedup vs baseline: 1.0638x; 1.0015x over previous
"""EnsembleFC (E=16 MLPs, 512->512->512->1, relu) on 8 TRN2 NeuronCores.

Strategy (expert parallel): each core owns E/8 = 2 ensemble members' weights
and computes their [B] output column; x is replicated. All activations stay
in "feature-major" (transposed) layout so no on-device transposes are needed:

    h1^T = relu(W1^T @ x^T + b1)      [H, B]
    h2^T = relu(W2^T @ h1^T + b2)     [H, B]
    out^T = W3^T @ h2^T               [1, B]   (b3 added on host)

Matmuls run in float32r (TRN2 reduced-precision fp32 PE mode, 1 cycle/row --
4x faster than plain fp32, ~20x more accurate than bf16; measured scaled
error ~1.5e-4 per 128-deep contraction with raw fp32 inputs).

Raw Bass (no Tile framework): this container's walrus rejects instructions
with more than a couple of sync waits, which Tile's auto-generated drains
exceed. Explicit per-engine programs with standalone waits keep every
instruction at one wait.

Pipeline per chunk of 512 batch columns (PE order, software-pipelined):
  ... L1(c,m0) L1(c,m1) L3(c-1,m0) L3(c-1,m1) L2(c,m0) L2(c,m1) ...
  PSUM: each member-layer pair owns 2 banks (mt % 2 rotation); L3 reuses the
       member's first L2 bank at partition 0.
  ACT: relu+bias drains psum into h1/h2 (f32r).
  DVE: reduces h2 over k-tiles with the w3 weights in exact fp32
       (t_r = sum_kt w3[kt] * h2[kt], rounded to f32r at the end), so L3 is a
       single ones-vector matmul per member-chunk instead of four; also
       copies L3 psum rows to the output staging buffer.
  SP:  weight DMAs (per-tensor sems, split per k-tile), x chunk DMAs
       (per-slot sems -- DMA queue completions are unordered), output stores.
A short burst of dummy matmuls on scratch SBUF during the DMA prologue keeps
the PE HAM clock-gate warm so chunk 0 runs at full clock.
"""
import numpy as np

E, D, H, B = 16, 512, 512, 8192
N_CORES = 8
MPC = E // N_CORES          # members per core
KT = D // 128               # k-tiles per 512 contraction
MT = H // 128               # m-tiles per 512 output dim
CH = 512                    # batch columns per chunk (one psum bank)
NCH = B // CH               # chunks
XBUF = 4                    # x chunk buffering

_CACHE = {}


def _build():
    import concourse.bass as bass
    from concourse import mybir

    f32 = mybir.dt.float32
    f32r = mybir.dt.float32r

    nc = bass.Bass("TRN2", target_bir_lowering=False, debug=False,
                   num_devices=N_CORES)

    xT = nc.dram_tensor("xT", [D, B], f32r, kind="ExternalInput").ap()
    w1 = nc.dram_tensor("w1", [MPC, D, H], f32r, kind="ExternalInput").ap()
    w2 = nc.dram_tensor("w2", [MPC, H, H], f32r, kind="ExternalInput").ap()
    # host-side pre-arranged: w3[p, m, kt], b1/b2[p, m, mt]
    w3 = nc.dram_tensor("w3", [128, MPC, KT], f32r, kind="ExternalInput").ap()
    b1 = nc.dram_tensor("b1", [128, MPC, MT], f32, kind="ExternalInput").ap()
    b2 = nc.dram_tensor("b2", [128, MPC, MT], f32, kind="ExternalInput").ap()
    one = nc.dram_tensor("one", [128, 1], f32r, kind="ExternalInput").ap()
    out = nc.dram_tensor("out", [MPC, B], f32, kind="ExternalOutput").ap()

    w1s = [nc.alloc_sbuf_tensor(f"w1s{m}", [128, KT, H], f32r).ap()
           for m in range(MPC)]
    w2s = [nc.alloc_sbuf_tensor(f"w2s{m}", [128, KT, H], f32r).ap()
           for m in range(MPC)]
    w3s = nc.alloc_sbuf_tensor("w3s", [128, MPC, KT], f32r).ap()
    b1s = nc.alloc_sbuf_tensor("b1s", [128, MPC, MT], f32).ap()
    b2s = nc.alloc_sbuf_tensor("b2s", [128, MPC, MT], f32).ap()
    ones_s = nc.alloc_sbuf_tensor("ones_s", [128, 1], f32r).ap()
    xs = nc.alloc_sbuf_tensor("xs", [128, XBUF, KT, CH], f32r).ap()
    h1 = nc.alloc_sbuf_tensor("h1", [128, MPC, KT, CH], f32r).ap()
    h2 = nc.alloc_sbuf_tensor("h2", [128, MPC, KT, CH], f32r).ap()
    # DVE kt-reduction scratch (no aliasing: A,B pair-products, C,D partials)
    rA = nc.alloc_sbuf_tensor("rA", [128, CH], f32).ap()
    rB = nc.alloc_sbuf_tensor("rB", [128, CH], f32).ap()
    rC = nc.alloc_sbuf_tensor("rC", [128, CH], f32).ap()
    rD = nc.alloc_sbuf_tensor("rD", [128, CH], f32).ap()
    t_r = nc.alloc_sbuf_tensor("t_r", [128, MPC, CH], f32r).ap()
    # per-member output staging, both at partition 0
    osb = [nc.alloc_sbuf_tensor(f"osb{m}", [1, NCH, CH], f32).ap()
           for m in range(MPC)]

    psA = nc.alloc_psum_tensor("psA", [128, 2 * MPC, CH], f32).ap()  # L1
    psB = nc.alloc_psum_tensor("psB", [128, 2 * MPC, CH], f32).ap()  # L2+L3

    # PE warmup scratch: dummy matmuls during the DMA prologue keep the HAM
    # clock-gate ramp off the critical path (uninitialized on HW -- harmless)
    scr = nc.alloc_sbuf_tensor("scr", [128, 128 + CH], f32r).ap()
    N_WARM = _CACHE.get("n_warm_override", 28)

    xT_r = xT.rearrange("(kt p) b -> p kt b", p=128)

    # --- tick tables (absolute semaphore counts, mirror emission order) ---
    mmT = {}
    _t = 0
    for c in range(NCH):
        for m in range(MPC):
            for mt in range(MT):
                _t += 1
                mmT[("l1", c, m, mt)] = _t
        if c >= 1:
            for m in range(MPC):
                _t += 1
                mmT[("l3", c - 1, m)] = _t
        for m in range(MPC):
            for mt in range(MT):
                _t += 1
                mmT[("l2", c, m, mt)] = _t
    for m in range(MPC):
        _t += 1
        mmT[("l3", NCH - 1, m)] = _t

    def act_r1(c, m, mt):
        return 16 * c + 4 * m + mt + 1

    def act_r2(c, m, mt):
        return 16 * c + 8 + 4 * m + mt + 1

    def dve_red(c, m):
        return 4 * c + m + 1

    def dve_cp(c, m):
        return 4 * c + 2 + m + 1

    with (
        nc.Block() as block,
        nc.semaphore("mm_sem") as mm_sem,
        nc.semaphore("act_sem") as act_sem,
        nc.semaphore("b1_sem") as b1_sem,
        nc.semaphore("b2_sem") as b2_sem,
        nc.semaphore("w3_sem") as w3_sem,
        nc.semaphore("d_sem") as d_sem,
    ):
        # per-slot x semaphores: DMA queue completions are unordered across
        # chunks, so a single cumulative counter would be racy
        x_sems = [nc.alloc_semaphore(f"x_sem{s}") for s in range(XBUF)]
        dve_sem = nc.alloc_semaphore("dve_sem")
        rd_sem = nc.alloc_semaphore("rd_sem")   # intra-DVE RAW/WAR ordering
        w1_sems = [nc.alloc_semaphore(f"w1_sem{m}") for m in range(MPC)]
        w2_sems = [nc.alloc_semaphore(f"w2_sem{m}") for m in range(MPC)]

        def dma_x(sync, c):
            for kt in range(KT):
                sync.dma_start(
                    out=xs[:, c % XBUF, kt, :],
                    in_=xT_r[:, kt, c * CH:(c + 1) * CH],
                ).then_inc(x_sems[c % XBUF], 16)

        @block.sync
        def _(sync: bass.BassEngine):
            # interleave weight loads with early x chunks, ordered by need
            w1r = [w1[m].rearrange("(kt p) m2 -> p kt m2", p=128)
                   for m in range(MPC)]
            w2r = [w2[m].rearrange("(kt p) m2 -> p kt m2", p=128)
                   for m in range(MPC)]
            for kt in range(KT):
                sync.dma_start(out=w1s[0][:, kt], in_=w1r[0][:, kt]
                               ).then_inc(w1_sems[0], 16)
            sync.dma_start(out=b1s, in_=b1).then_inc(b1_sem, 16)
            dma_x(sync, 0)
            for kt in range(KT):
                sync.dma_start(out=w1s[1][:, kt], in_=w1r[1][:, kt]
                               ).then_inc(w1_sems[1], 16)
            sync.dma_start(out=b2s, in_=b2).then_inc(b2_sem, 16)
            sync.dma_start(out=w3s, in_=w3).then_inc(w3_sem, 16)
            sync.dma_start(out=ones_s, in_=one).then_inc(w3_sem, 16)
            dma_x(sync, 1)
            for kt in range(KT):
                sync.dma_start(out=w2s[0][:, kt], in_=w2r[0][:, kt]
                               ).then_inc(w2_sems[0], 16)
            dma_x(sync, 2)
            for kt in range(KT):
                sync.dma_start(out=w2s[1][:, kt], in_=w2r[1][:, kt]
                               ).then_inc(w2_sems[1], 16)
            dma_x(sync, 3)

            out_r = out.rearrange("m (nch ch) -> m nch ch", ch=CH)
            for c in range(XBUF, NCH):
                # x slot free once L1 of chunk c-XBUF fully consumed it
                sync.wait_ge(mm_sem, mmT[("l1", c - XBUF, MPC - 1, MT - 1)])
                dma_x(sync, c)
                # trailing store for chunk c-XBUF
                cs = c - XBUF
                sync.wait_ge(dve_sem, dve_cp(cs, MPC - 1))
                for m in range(MPC):
                    sync.dma_start(out=out_r[m:m + 1, cs],
                                   in_=osb[m][:, cs]).then_inc(d_sem, 16)

            for cs in range(NCH - XBUF, NCH):
                sync.wait_ge(dve_sem, dve_cp(cs, MPC - 1))
                for m in range(MPC):
                    sync.dma_start(out=out_r[m:m + 1, cs],
                                   in_=osb[m][:, cs]).then_inc(d_sem, 16)
            sync.wait_ge(d_sem, 16 * MPC * NCH)

        @block.vector
        def _(vector: bass.BassEngine):
            # DVE: (a) kt-reduction t_r = sum_kt w3[kt]*h2[kt] in exact fp32
            # (takes 3 of every 4 L3 matmuls off the PE, and is more accurate
            # than f32r products), (b) L3 psum -> osb copies.
            w3f = w3s.bitcast(f32)
            for c in range(NCH):
                for m in range(MPC):
                    # h2 ready; implies PE already read t_r(c-1, m) (its L3
                    # precedes this chunk's L2 in the PE stream)
                    vector.wait_ge(act_sem, act_r2(c, m, MT - 1))
                    h2f = h2[:, m].bitcast(f32)
                    # DVE does not self-interlock same-engine RAW/WAR;
                    # rd_sem orders the reduction chain explicitly
                    base = 6 * (MPC * c + m)
                    if base:
                        vector.wait_ge(rd_sem, base)
                    vector.tensor_scalar_mul(rA, h2f[:, 0, :], w3f[:, m, 0:1]
                                             ).then_inc(rd_sem, 1)
                    vector.tensor_scalar_mul(rB, h2f[:, 1, :], w3f[:, m, 1:2]
                                             ).then_inc(rd_sem, 1)
                    vector.wait_ge(rd_sem, base + 2)
                    vector.tensor_add(rC, rA, rB).then_inc(rd_sem, 1)
                    vector.wait_ge(rd_sem, base + 3)   # addC done before rA/rB reuse
                    vector.tensor_scalar_mul(rA, h2f[:, 2, :], w3f[:, m, 2:3]
                                             ).then_inc(rd_sem, 1)
                    vector.tensor_scalar_mul(rB, h2f[:, 3, :], w3f[:, m, 3:4]
                                             ).then_inc(rd_sem, 1)
                    vector.wait_ge(rd_sem, base + 5)
                    vector.tensor_add(rD, rA, rB).then_inc(rd_sem, 1)
                    vector.wait_ge(rd_sem, base + 6)
                    vector.tensor_add(t_r[:, m, :], rC, rD
                                      ).then_inc(dve_sem, 1)
                for m in range(MPC):
                    vector.wait_ge(mm_sem, mmT[("l3", c, m)])
                    vector.tensor_copy(
                        osb[m][0:1, c, :], psB[0:1, 2 * m + 1, :],
                    ).then_inc(dve_sem, 1)

        @block.tensor
        def _(tensor: bass.BassEngine):
            # warmup on uninitialized scratch: values are irrelevant, the psum
            # is overwritten (start=True) before any reader
            for i in range(N_WARM):
                tensor.matmul(psA[:, 0, :], scr[:, :128], scr[:, 128:],
                              start=True, stop=True, skip_group_check=True)

            def l3(c, m):
                # single ones-matmul over the DVE-reduced t_r; bank 2m+1 so
                # the osb copy only gates the SECOND L2 group of chunk c+1
                tensor.wait_ge(dve_sem, dve_red(c, m))
                tensor.matmul(
                    psB[0:1, 2 * m + 1, :], ones_s, t_r[:, m, :],
                    start=True, stop=True,
                ).then_inc(mm_sem, 1)

            for c in range(NCH):
                tensor.wait_ge(x_sems[c % XBUF], 64 * (c // XBUF + 1))
                # L1 both members
                for m in range(MPC):
                    if c == 0:
                        tensor.wait_ge(w1_sems[m], 64)
                    for mt in range(MT):
                        if mt >= 2:           # 2-bank rotation WAR
                            tensor.wait_ge(act_sem, act_r1(c, m, mt - 2))
                        elif c > 0:           # bank last used by c-1, mt+2
                            tensor.wait_ge(act_sem, act_r1(c - 1, m, mt + 2))
                        for kt in range(KT):
                            ins = tensor.matmul(
                                psA[:, 2 * m + mt % 2, :],
                                w1s[m][:, kt, mt * 128:(mt + 1) * 128],
                                xs[:, c % XBUF, kt, :],
                                start=(kt == 0), stop=(kt == KT - 1),
                            )
                        ins.then_inc(mm_sem, 1)
                # pipelined L3 of the previous chunk: its DVE reduction ran
                # while this chunk's L1 was on the PE
                if c >= 1:
                    if c == 1:
                        tensor.wait_ge(w3_sem, 32)
                    for m in range(MPC):
                        l3(c - 1, m)
                # L2 both members
                for m in range(MPC):
                    if c == 0:
                        tensor.wait_ge(w2_sems[m], 64)
                    tensor.wait_ge(act_sem, act_r1(c, m, MT - 1))  # h1 ready
                    for mt in range(MT):
                        if mt == 1 and c > 0:
                            # psB bank 2m+1 holds chunk c-1's L3 row until
                            # DVE copies it out
                            tensor.wait_ge(dve_sem, dve_cp(c - 1, m))
                        if mt >= 2:
                            tensor.wait_ge(act_sem, act_r2(c, m, mt - 2))
                        for kt in range(KT):
                            ins = tensor.matmul(
                                psB[:, 2 * m + mt % 2, :],
                                w2s[m][:, kt, mt * 128:(mt + 1) * 128],
                                h1[:, m, kt, :],
                                start=(kt == 0), stop=(kt == KT - 1),
                            )
                        ins.then_inc(mm_sem, 1)
            # tail: direct w3 matmuls for the last chunk -- avoids idling on
            # the serial DVE reduction after the final L2 (DVE still emits
            # red(NCH-1) for tick consistency; its t_r is simply unused)
            for m in range(MPC):
                tensor.wait_ge(act_sem, act_r2(NCH - 1, m, MT - 1))
                for kt in range(KT):
                    ins = tensor.matmul(
                        psB[0:1, 2 * m + 1, :],
                        w3s[:, m, kt:kt + 1],
                        h2[:, m, kt, :],
                        start=(kt == 0), stop=(kt == KT - 1),
                    )
                ins.then_inc(mm_sem, 1)

        @block.scalar
        def _(scalar: bass.BassEngine):
            Relu = bass.mybir.ActivationFunctionType.Relu
            scalar.wait_ge(b1_sem, 16)
            scalar.wait_ge(b2_sem, 16)
            for c in range(NCH):
                for m in range(MPC):
                    for mt in range(MT):
                        scalar.wait_ge(mm_sem, mmT[("l1", c, m, mt)])
                        scalar.activation(
                            h1[:, m, mt, :], psA[:, 2 * m + mt % 2, :], Relu,
                            bias=b1s[:, m, mt:mt + 1],
                        ).then_inc(act_sem, 1)
                for m in range(MPC):
                    for mt in range(MT):
                        scalar.wait_ge(mm_sem, mmT[("l2", c, m, mt)])
                        scalar.activation(
                            h2[:, m, mt, :], psB[:, 2 * m + mt % 2, :], Relu,
                            bias=b2s[:, m, mt:mt + 1],
                        ).then_inc(act_sem, 1)

    return nc


def get_nc():
    if "nc" not in _CACHE:
        _CACHE["nc"] = _build()
    return _CACHE["nc"]


def kernel(x, W1, b1, W2, b2, W3, b3):
    from concourse.bass_utils import run_bass_kernel_spmd

    nc = get_nc()
    xT = np.ascontiguousarray(np.asarray(x, dtype=np.float32).T)
    W1 = np.asarray(W1, dtype=np.float32)
    W2 = np.asarray(W2, dtype=np.float32)
    W3 = np.asarray(W3, dtype=np.float32)
    b1 = np.asarray(b1, dtype=np.float32)
    b2 = np.asarray(b2, dtype=np.float32)
    b3 = np.asarray(b3, dtype=np.float32)

    def feat_major(v):
        # [MPC, H] -> [128, MPC, H//128]: v[p, m, t] = v_in[m, t*128 + p]
        return np.ascontiguousarray(
            v.reshape(MPC, H // 128, 128).transpose(2, 0, 1))

    in_maps = []
    for c in range(N_CORES):
        s = slice(MPC * c, MPC * (c + 1))
        in_maps.append({
            "xT": xT,
            "w1": np.ascontiguousarray(W1[s]),
            "w2": np.ascontiguousarray(W2[s]),
            "w3": feat_major(W3[s, :, 0]),
            "b1": feat_major(b1[s]),
            "b2": feat_major(b2[s]),
            "one": np.ones((128, 1), dtype=np.float32),
        })

    res = run_bass_kernel_spmd(nc, in_maps, list(range(N_CORES)))
    out = np.concatenate([r["out"] for r in res.results], axis=0)  # [E, B]
    out = out + b3.reshape(E, 1)
    return out.reshape(E, B, 1).astype(np.float32)


# revision 42
# speedup vs baseline: 1.0806x; 1.0158x over previous
"""EnsembleFC (E=16 MLPs, 512->512->512->1, relu) on 8 TRN2 NeuronCores.

Strategy (expert parallel): each core owns E/8 = 2 ensemble members' weights
and computes their [B] output column; x is replicated. All activations stay
in "feature-major" (transposed) layout so no on-device transposes are needed:

    h1^T = relu(W1^T @ x^T + b1)      [H, B]
    h2^T = relu(W2^T @ h1^T + b2)     [H, B]
    out^T = W3^T @ h2^T               [1, B]   (b3 added on host)

Matmuls run in float32r (TRN2 reduced-precision fp32 PE mode, 1 cycle/row --
4x faster than plain fp32, ~20x more accurate than bf16; measured scaled
error ~1.5e-4 per 128-deep contraction with raw fp32 inputs).

Raw Bass (no Tile framework): this container's walrus rejects instructions
with more than a couple of sync waits, which Tile's auto-generated drains
exceed. Explicit per-engine programs with standalone waits keep every
instruction at one wait.

Pipeline per chunk of 512 batch columns (PE order, software-pipelined):
  ... L1(c,m0) L1(c,m1) L3(c-1,m0) L3(c-1,m1) L2(c,m0) L2(c,m1) ...
  PSUM: each member-layer pair owns 2 banks (mt % 2 rotation); L3 reuses the
       member's first L2 bank at partition 0.
  ACT: relu+bias drains psum into h1/h2 (f32r).
  DVE: reduces h2 over k-tiles with the w3 weights in exact fp32
       (t_r = sum_kt w3[kt] * h2[kt], rounded to f32r at the end), so L3 is a
       single ones-vector matmul per member-chunk instead of four; also
       copies L3 psum rows to the output staging buffer.
  SP:  weight DMAs (per-tensor sems, split per k-tile), x chunk DMAs
       (per-slot sems -- DMA queue completions are unordered), output stores.
A short burst of dummy matmuls on scratch SBUF during the DMA prologue keeps
the PE HAM clock-gate warm so chunk 0 runs at full clock.
"""
import numpy as np

E, D, H, B = 16, 512, 512, 8192
N_CORES = 8
MPC = E // N_CORES          # members per core
KT = D // 128               # k-tiles per 512 contraction
MT = H // 128               # m-tiles per 512 output dim
CH = 512                    # batch columns per chunk (one psum bank)
NCH = B // CH               # chunks
XBUF = 4                    # x chunk buffering

_CACHE = {}


def _build():
    import concourse.bass as bass
    from concourse import mybir

    f32 = mybir.dt.float32
    f32r = mybir.dt.float32r

    nc = bass.Bass("TRN2", target_bir_lowering=False, debug=False,
                   num_devices=N_CORES)

    xT = nc.dram_tensor("xT", [D, B], f32r, kind="ExternalInput").ap()
    w1 = nc.dram_tensor("w1", [MPC, D, H], f32r, kind="ExternalInput").ap()
    w2 = nc.dram_tensor("w2", [MPC, H, H], f32r, kind="ExternalInput").ap()
    # host-side pre-arranged: w3[p, m, kt], b1/b2[p, m, mt]
    w3 = nc.dram_tensor("w3", [128, MPC, KT], f32r, kind="ExternalInput").ap()
    b1 = nc.dram_tensor("b1", [128, MPC, MT], f32, kind="ExternalInput").ap()
    b2 = nc.dram_tensor("b2", [128, MPC, MT], f32, kind="ExternalInput").ap()
    one = nc.dram_tensor("one", [128, 1], f32r, kind="ExternalInput").ap()
    out = nc.dram_tensor("out", [MPC, B], f32, kind="ExternalOutput").ap()

    w1s = [nc.alloc_sbuf_tensor(f"w1s{m}", [128, KT, H], f32r).ap()
           for m in range(MPC)]
    w2s = [nc.alloc_sbuf_tensor(f"w2s{m}", [128, KT, H], f32r).ap()
           for m in range(MPC)]
    w3s = nc.alloc_sbuf_tensor("w3s", [128, MPC, KT], f32r).ap()
    b1s = nc.alloc_sbuf_tensor("b1s", [128, MPC, MT], f32).ap()
    b2s = nc.alloc_sbuf_tensor("b2s", [128, MPC, MT], f32).ap()
    ones_s = nc.alloc_sbuf_tensor("ones_s", [128, 1], f32r).ap()
    xs = nc.alloc_sbuf_tensor("xs", [128, XBUF, KT, CH], f32r).ap()
    h1 = nc.alloc_sbuf_tensor("h1", [128, MPC, KT, CH], f32r).ap()
    h2 = nc.alloc_sbuf_tensor("h2", [128, MPC, KT, CH], f32r).ap()
    # DVE kt-reduction scratch (no aliasing: A,B pair-products, C,D partials)
    rA = nc.alloc_sbuf_tensor("rA", [128, CH], f32).ap()
    rB = nc.alloc_sbuf_tensor("rB", [128, CH], f32).ap()
    rC = nc.alloc_sbuf_tensor("rC", [128, CH], f32).ap()
    rD = nc.alloc_sbuf_tensor("rD", [128, CH], f32).ap()
    t_r = nc.alloc_sbuf_tensor("t_r", [128, MPC, CH], f32r).ap()
    # per-member output staging, both at partition 0
    osb = [nc.alloc_sbuf_tensor(f"osb{m}", [1, NCH, CH], f32).ap()
           for m in range(MPC)]

    psA = nc.alloc_psum_tensor("psA", [128, 2 * MPC, CH], f32).ap()  # L1
    psB = nc.alloc_psum_tensor("psB", [128, 2 * MPC, CH], f32).ap()  # L2+L3

    # PE warmup scratch: dummy matmuls during the DMA prologue keep the HAM
    # clock-gate ramp off the critical path (uninitialized on HW -- harmless)
    scr = nc.alloc_sbuf_tensor("scr", [128, 128 + CH], f32r).ap()
    N_WARM = _CACHE.get("n_warm_override", 28)

    xT_r = xT.rearrange("(kt p) b -> p kt b", p=128)

    # --- tick tables (absolute semaphore counts, mirror emission order) ---
    mmT = {}
    _t = 0
    for c in range(NCH):
        for m in range(MPC):
            for mt in range(MT):
                _t += 1
                mmT[("l1", c, m, mt)] = _t
        if c >= 1:
            for m in range(MPC):
                _t += 1
                mmT[("l3", c - 1, m)] = _t
        for m in range(MPC):
            for mt in range(MT):
                _t += 1
                mmT[("l2", c, m, mt)] = _t
    for m in range(MPC):
        _t += 1
        mmT[("l3", NCH - 1, m)] = _t

    def act_r1(c, m, mt):
        return 16 * c + 4 * m + mt + 1

    def act_r2(c, m, mt):
        return 16 * c + 8 + 4 * m + mt + 1

    # DVE tick table: per chunk red(m0), red(m1) [skipped for the last
    # chunk -- its L3 runs directly off h2], then cp(m0), cp(m1)
    dveT = {}
    _d = 0
    for c in range(NCH):
        if c < NCH - 1:
            for m in range(MPC):
                _d += 1
                dveT[("red", c, m)] = _d
        for m in range(MPC):
            _d += 1
            dveT[("cp", c, m)] = _d

    def dve_red(c, m):
        return dveT[("red", c, m)]

    def dve_cp(c, m):
        return dveT[("cp", c, m)]

    with (
        nc.Block() as block,
        nc.semaphore("mm_sem") as mm_sem,
        nc.semaphore("act_sem") as act_sem,
        nc.semaphore("b1_sem") as b1_sem,
        nc.semaphore("b2_sem") as b2_sem,
        nc.semaphore("w3_sem") as w3_sem,
        nc.semaphore("d_sem") as d_sem,
    ):
        # per-slot x semaphores: DMA queue completions are unordered across
        # chunks, so a single cumulative counter would be racy
        x_sems = [nc.alloc_semaphore(f"x_sem{s}") for s in range(XBUF)]
        dve_sem = nc.alloc_semaphore("dve_sem")
        rd_sem = nc.alloc_semaphore("rd_sem")   # intra-DVE RAW/WAR ordering
        w1_sems = [nc.alloc_semaphore(f"w1_sem{m}") for m in range(MPC)]
        w2_sems = [nc.alloc_semaphore(f"w2_sem{m}") for m in range(MPC)]

        def dma_x(sync, c):
            for kt in range(KT):
                sync.dma_start(
                    out=xs[:, c % XBUF, kt, :],
                    in_=xT_r[:, kt, c * CH:(c + 1) * CH],
                ).then_inc(x_sems[c % XBUF], 16)

        @block.sync
        def _(sync: bass.BassEngine):
            # interleave weight loads with early x chunks, ordered by need
            w1r = [w1[m].rearrange("(kt p) m2 -> p kt m2", p=128)
                   for m in range(MPC)]
            w2r = [w2[m].rearrange("(kt p) m2 -> p kt m2", p=128)
                   for m in range(MPC)]
            for kt in range(KT):
                sync.dma_start(out=w1s[0][:, kt], in_=w1r[0][:, kt]
                               ).then_inc(w1_sems[0], 16)
            sync.dma_start(out=b1s, in_=b1).then_inc(b1_sem, 16)
            dma_x(sync, 0)
            for kt in range(KT):
                sync.dma_start(out=w1s[1][:, kt], in_=w1r[1][:, kt]
                               ).then_inc(w1_sems[1], 16)
            sync.dma_start(out=b2s, in_=b2).then_inc(b2_sem, 16)
            sync.dma_start(out=w3s, in_=w3).then_inc(w3_sem, 16)
            sync.dma_start(out=ones_s, in_=one).then_inc(w3_sem, 16)
            dma_x(sync, 1)
            for kt in range(KT):
                sync.dma_start(out=w2s[0][:, kt], in_=w2r[0][:, kt]
                               ).then_inc(w2_sems[0], 16)
            dma_x(sync, 2)
            for kt in range(KT):
                sync.dma_start(out=w2s[1][:, kt], in_=w2r[1][:, kt]
                               ).then_inc(w2_sems[1], 16)
            dma_x(sync, 3)

            out_r = out.rearrange("m (nch ch) -> m nch ch", ch=CH)
            for c in range(XBUF, NCH):
                # x slot free once L1 of chunk c-XBUF fully consumed it
                sync.wait_ge(mm_sem, mmT[("l1", c - XBUF, MPC - 1, MT - 1)])
                dma_x(sync, c)
                # trailing store for chunk c-XBUF
                cs = c - XBUF
                sync.wait_ge(dve_sem, dve_cp(cs, MPC - 1))
                for m in range(MPC):
                    sync.dma_start(out=out_r[m:m + 1, cs],
                                   in_=osb[m][:, cs]).then_inc(d_sem, 16)

            for cs in range(NCH - XBUF, NCH):
                sync.wait_ge(dve_sem, dve_cp(cs, MPC - 1))
                for m in range(MPC):
                    sync.dma_start(out=out_r[m:m + 1, cs],
                                   in_=osb[m][:, cs]).then_inc(d_sem, 16)
            sync.wait_ge(d_sem, 16 * MPC * NCH)

        @block.vector
        def _(vector: bass.BassEngine):
            # DVE: (a) kt-reduction t_r = sum_kt w3[kt]*h2[kt] in exact fp32
            # (takes 3 of every 4 L3 matmuls off the PE, and is more accurate
            # than f32r products), (b) L3 psum -> osb copies.
            w3f = w3s.bitcast(f32)
            for c in range(NCH):
                for m in range(MPC):
                    if c == NCH - 1:
                        break   # last chunk: PE computes L3 directly
                    # h2 ready; implies PE already read t_r(c-1, m) (its L3
                    # precedes this chunk's L2 in the PE stream)
                    vector.wait_ge(act_sem, act_r2(c, m, MT - 1))
                    h2f = h2[:, m].bitcast(f32)
                    # DVE does not self-interlock same-engine RAW/WAR;
                    # rd_sem orders the reduction chain explicitly
                    base = 6 * (MPC * c + m)
                    if base:
                        vector.wait_ge(rd_sem, base)
                    vector.tensor_scalar_mul(rA, h2f[:, 0, :], w3f[:, m, 0:1]
                                             ).then_inc(rd_sem, 1)
                    vector.tensor_scalar_mul(rB, h2f[:, 1, :], w3f[:, m, 1:2]
                                             ).then_inc(rd_sem, 1)
                    vector.wait_ge(rd_sem, base + 2)
                    vector.tensor_add(rC, rA, rB).then_inc(rd_sem, 1)
                    vector.wait_ge(rd_sem, base + 3)   # addC done before rA/rB reuse
                    vector.tensor_scalar_mul(rA, h2f[:, 2, :], w3f[:, m, 2:3]
                                             ).then_inc(rd_sem, 1)
                    vector.tensor_scalar_mul(rB, h2f[:, 3, :], w3f[:, m, 3:4]
                                             ).then_inc(rd_sem, 1)
                    vector.wait_ge(rd_sem, base + 5)
                    vector.tensor_add(rD, rA, rB).then_inc(rd_sem, 1)
                    vector.wait_ge(rd_sem, base + 6)
                    vector.tensor_add(t_r[:, m, :], rC, rD
                                      ).then_inc(dve_sem, 1)
                for m in range(MPC):
                    vector.wait_ge(mm_sem, mmT[("l3", c, m)])
                    vector.tensor_copy(
                        osb[m][0:1, c, :], psB[0:1, 2 * m + 1, :],
                    ).then_inc(dve_sem, 1)

        @block.tensor
        def _(tensor: bass.BassEngine):
            # warmup on uninitialized scratch: values are irrelevant, the psum
            # is overwritten (start=True) before any reader
            for i in range(N_WARM):
                tensor.matmul(psA[:, 0, :], scr[:, :128], scr[:, 128:],
                              start=True, stop=True, skip_group_check=True)

            def l3(c, m):
                # single ones-matmul over the DVE-reduced t_r; bank 2m+1 so
                # the osb copy only gates the SECOND L2 group of chunk c+1
                tensor.wait_ge(dve_sem, dve_red(c, m))
                tensor.matmul(
                    psB[0:1, 2 * m + 1, :], ones_s, t_r[:, m, :],
                    start=True, stop=True,
                ).then_inc(mm_sem, 1)

            for c in range(NCH):
                tensor.wait_ge(x_sems[c % XBUF], 64 * (c // XBUF + 1))
                # L1 both members
                for m in range(MPC):
                    if c == 0:
                        tensor.wait_ge(w1_sems[m], 64)
                    for mt in range(MT):
                        if mt >= 2:           # 2-bank rotation WAR
                            tensor.wait_ge(act_sem, act_r1(c, m, mt - 2))
                        elif c > 0:           # bank last used by c-1, mt+2
                            tensor.wait_ge(act_sem, act_r1(c - 1, m, mt + 2))
                        for kt in range(KT):
                            ins = tensor.matmul(
                                psA[:, 2 * m + mt % 2, :],
                                w1s[m][:, kt, mt * 128:(mt + 1) * 128],
                                xs[:, c % XBUF, kt, :],
                                start=(kt == 0), stop=(kt == KT - 1),
                            )
                        ins.then_inc(mm_sem, 1)
                # pipelined L3 of the previous chunk: its DVE reduction ran
                # while this chunk's L1 was on the PE
                if c >= 1:
                    if c == 1:
                        tensor.wait_ge(w3_sem, 32)
                    for m in range(MPC):
                        l3(c - 1, m)
                # L2 both members
                for m in range(MPC):
                    if c == 0:
                        tensor.wait_ge(w2_sems[m], 64)
                    tensor.wait_ge(act_sem, act_r1(c, m, MT - 1))  # h1 ready
                    for mt in range(MT):
                        if mt == 1 and c > 0:
                            # psB bank 2m+1 holds chunk c-1's L3 row until
                            # DVE copies it out
                            tensor.wait_ge(dve_sem, dve_cp(c - 1, m))
                        if mt >= 2:
                            tensor.wait_ge(act_sem, act_r2(c, m, mt - 2))
                        for kt in range(KT):
                            ins = tensor.matmul(
                                psB[:, 2 * m + mt % 2, :],
                                w2s[m][:, kt, mt * 128:(mt + 1) * 128],
                                h1[:, m, kt, :],
                                start=(kt == 0), stop=(kt == KT - 1),
                            )
                        ins.then_inc(mm_sem, 1)
            # tail: direct w3 matmuls for the last chunk -- avoids idling on
            # the serial DVE reduction after the final L2
            for m in range(MPC):
                tensor.wait_ge(act_sem, act_r2(NCH - 1, m, MT - 1))
                for kt in range(KT):
                    ins = tensor.matmul(
                        psB[0:1, 2 * m + 1, :],
                        w3s[:, m, kt:kt + 1],
                        h2[:, m, kt, :],
                        start=(kt == 0), stop=(kt == KT - 1),
                    )
                ins.then_inc(mm_sem, 1)

        @block.scalar
        def _(scalar: bass.BassEngine):
            Relu = bass.mybir.ActivationFunctionType.Relu
            scalar.wait_ge(b1_sem, 16)
            scalar.wait_ge(b2_sem, 16)
            for c in range(NCH):
                for m in range(MPC):
                    for mt in range(MT):
                        scalar.wait_ge(mm_sem, mmT[("l1", c, m, mt)])
                        scalar.activation(
                            h1[:, m, mt, :], psA[:, 2 * m + mt % 2, :], Relu,
                            bias=b1s[:, m, mt:mt + 1],
                        ).then_inc(act_sem, 1)
                for m in range(MPC):
                    for mt in range(MT):
                        scalar.wait_ge(mm_sem, mmT[("l2", c, m, mt)])
                        scalar.activation(
                            h2[:, m, mt, :], psB[:, 2 * m + mt % 2, :], Relu,
                            bias=b2s[:, m, mt:mt + 1],
                        ).then_inc(act_sem, 1)

    return nc


def get_nc():
    if "nc" not in _CACHE:
        _CACHE["nc"] = _build()
    return _CACHE["nc"]


def kernel(x, W1, b1, W2, b2, W3, b3):
    from concourse.bass_utils import run_bass_kernel_spmd

    nc = get_nc()
    xT = np.ascontiguousarray(np.asarray(x, dtype=np.float32).T)
    W1 = np.asarray(W1, dtype=np.float32)
    W2 = np.asarray(W2, dtype=np.float32)
    W3 = np.asarray(W3, dtype=np.float32)
    b1 = np.asarray(b1, dtype=np.float32)
    b2 = np.asarray(b2, dtype=np.float32)
    b3 = np.asarray(b3, dtype=np.float32)

    def feat_major(v):
        # [MPC, H] -> [128, MPC, H//128]: v[p, m, t] = v_in[m, t*128 + p]
        return np.ascontiguousarray(
            v.reshape(MPC, H // 128, 128).transpose(2, 0, 1))

    in_maps = []
    for c in range(N_CORES):
        s = slice(MPC * c, MPC * (c + 1))
        in_maps.append({
            "xT": xT,
            "w1": np.ascontiguousarray(W1[s]),
            "w2": np.ascontiguousarray(W2[s]),
            "w3": feat_major(W3[s, :, 0]),
            "b1": feat_major(b1[s]),
            "b2": feat_major(b2[s]),
            "one": np.ones((128, 1), dtype=np.float32),
        })

    res = run_bass_kernel_spmd(nc, in_maps, list(range(N_CORES)))
    out = np.concatenate([r["out"] for r in res.results], axis=0)  # [E, B]
    out = out + b3.reshape(E, 1)
    return out.reshape(E, B, 1).astype(np.float32)


# revision 45
# speedup vs baseline: 1.0821x; 1.0014x over previous
"""EnsembleFC (E=16 MLPs, 512->512->512->1, relu) on 8 TRN2 NeuronCores.

Strategy (expert parallel): each core owns E/8 = 2 ensemble members' weights
and computes their [B] output column; x is replicated. All activations stay
in "feature-major" (transposed) layout so no on-device transposes are needed:

    h1^T = relu(W1^T @ x^T + b1)      [H, B]
    h2^T = relu(W2^T @ h1^T + b2)     [H, B]
    out^T = W3^T @ h2^T               [1, B]   (b3 added on host)

Matmuls run in float32r (TRN2 reduced-precision fp32 PE mode, 1 cycle/row --
4x faster than plain fp32, ~20x more accurate than bf16; measured scaled
error ~1.5e-4 per 128-deep contraction with raw fp32 inputs).

Raw Bass (no Tile framework): this container's walrus rejects instructions
with more than a couple of sync waits, which Tile's auto-generated drains
exceed. Explicit per-engine programs with standalone waits keep every
instruction at one wait.

Pipeline per chunk of 512 batch columns (PE order, software-pipelined):
  ... L1(c,m0) L1(c,m1) L3(c-1,m0) L3(c-1,m1) L2(c,m0) L2(c,m1) ...
  PSUM: each member-layer pair owns 2 banks (mt % 2 rotation); L3 reuses the
       member's first L2 bank at partition 0.
  ACT: relu+bias drains psum into h1/h2 (f32r).
  DVE: reduces h2 over k-tiles with the w3 weights in exact fp32
       (t_r = sum_kt w3[kt] * h2[kt], rounded to f32r at the end), so L3 is a
       single ones-vector matmul per member-chunk instead of four; also
       copies L3 psum rows to the output staging buffer.
  SP:  weight DMAs (per-tensor sems, split per k-tile), x chunk DMAs
       (per-slot sems -- DMA queue completions are unordered), output stores.
A short burst of dummy matmuls on scratch SBUF during the DMA prologue keeps
the PE HAM clock-gate warm so chunk 0 runs at full clock.
"""
import numpy as np

E, D, H, B = 16, 512, 512, 8192
N_CORES = 8
MPC = E // N_CORES          # members per core
KT = D // 128               # k-tiles per 512 contraction
MT = H // 128               # m-tiles per 512 output dim
CH = 512                    # batch columns per chunk (one psum bank)
NCH = B // CH               # chunks
XBUF = 4                    # x chunk buffering

_CACHE = {}


def _build():
    import concourse.bass as bass
    from concourse import mybir

    f32 = mybir.dt.float32
    f32r = mybir.dt.float32r

    nc = bass.Bass("TRN2", target_bir_lowering=False, debug=False,
                   num_devices=N_CORES)

    xT = nc.dram_tensor("xT", [D, B], f32r, kind="ExternalInput").ap()
    w1 = nc.dram_tensor("w1", [MPC, D, H], f32r, kind="ExternalInput").ap()
    w2 = nc.dram_tensor("w2", [MPC, H, H], f32r, kind="ExternalInput").ap()
    # host-side pre-arranged: w3[p, m, kt], b1/b2[p, m, mt]
    w3 = nc.dram_tensor("w3", [128, MPC, KT], f32r, kind="ExternalInput").ap()
    b1 = nc.dram_tensor("b1", [128, MPC, MT], f32, kind="ExternalInput").ap()
    b2 = nc.dram_tensor("b2", [128, MPC, MT], f32, kind="ExternalInput").ap()
    one = nc.dram_tensor("one", [128, 1], f32r, kind="ExternalInput").ap()
    out = nc.dram_tensor("out", [MPC, B], f32, kind="ExternalOutput").ap()

    w1s = [nc.alloc_sbuf_tensor(f"w1s{m}", [128, KT, H], f32r).ap()
           for m in range(MPC)]
    w2s = [nc.alloc_sbuf_tensor(f"w2s{m}", [128, KT, H], f32r).ap()
           for m in range(MPC)]
    w3s = nc.alloc_sbuf_tensor("w3s", [128, MPC, KT], f32r).ap()
    b1s = nc.alloc_sbuf_tensor("b1s", [128, MPC, MT], f32).ap()
    b2s = nc.alloc_sbuf_tensor("b2s", [128, MPC, MT], f32).ap()
    ones_s = nc.alloc_sbuf_tensor("ones_s", [128, 1], f32r).ap()
    xs = nc.alloc_sbuf_tensor("xs", [128, XBUF, KT, CH], f32r).ap()
    h1 = nc.alloc_sbuf_tensor("h1", [128, MPC, KT, CH], f32r).ap()
    h2 = nc.alloc_sbuf_tensor("h2", [128, MPC, KT, CH], f32r).ap()
    # DVE kt-reduction scratch (no aliasing: A,B pair-products, C,D partials)
    rA = nc.alloc_sbuf_tensor("rA", [128, CH], f32).ap()
    rB = nc.alloc_sbuf_tensor("rB", [128, CH], f32).ap()
    rC = nc.alloc_sbuf_tensor("rC", [128, CH], f32).ap()
    rD = nc.alloc_sbuf_tensor("rD", [128, CH], f32).ap()
    t_r = nc.alloc_sbuf_tensor("t_r", [128, MPC, CH], f32r).ap()
    # per-member output staging, both at partition 0
    osb = [nc.alloc_sbuf_tensor(f"osb{m}", [1, NCH, CH], f32).ap()
           for m in range(MPC)]

    psA = nc.alloc_psum_tensor("psA", [128, 2 * MPC, CH], f32).ap()  # L1
    psB = nc.alloc_psum_tensor("psB", [128, 2 * MPC, CH], f32).ap()  # L2+L3

    # PE warmup scratch: dummy matmuls during the DMA prologue keep the HAM
    # clock-gate ramp off the critical path (uninitialized on HW -- harmless)
    scr = nc.alloc_sbuf_tensor("scr", [128, 128 + CH], f32r).ap()
    N_WARM = _CACHE.get("n_warm_override", 28)

    xT_r = xT.rearrange("(kt p) b -> p kt b", p=128)

    # --- tick tables (absolute semaphore counts, mirror emission order) ---
    mmT = {}
    _t = 0
    for c in range(NCH):
        for m in range(MPC):
            for mt in range(MT):
                _t += 1
                mmT[("l1", c, m, mt)] = _t
        if c >= 1:
            for m in range(MPC):
                _t += 1
                mmT[("l3", c - 1, m)] = _t
        for m in range(MPC):
            for mt in range(MT):
                _t += 1
                mmT[("l2", c, m, mt)] = _t
    for m in range(MPC):
        _t += 1
        mmT[("l3", NCH - 1, m)] = _t

    def act_r1(c, m, mt):
        return 16 * c + 4 * m + mt + 1

    def act_r2(c, m, mt):
        return 16 * c + 8 + 4 * m + mt + 1

    # DVE tick table: per chunk red(m0), red(m1) [skipped for the last
    # chunk -- its L3 runs directly off h2], then cp(m0), cp(m1)
    dveT = {}
    _d = 0
    for c in range(NCH):
        if c < NCH - 1:
            for m in range(MPC):
                _d += 1
                dveT[("red", c, m)] = _d
        for m in range(MPC):
            _d += 1
            dveT[("cp", c, m)] = _d

    def dve_red(c, m):
        return dveT[("red", c, m)]

    def dve_cp(c, m):
        return dveT[("cp", c, m)]

    with (
        nc.Block() as block,
        nc.semaphore("mm_sem") as mm_sem,
        nc.semaphore("act_sem") as act_sem,
        nc.semaphore("b1_sem") as b1_sem,
        nc.semaphore("b2_sem") as b2_sem,
        nc.semaphore("w3_sem") as w3_sem,
        nc.semaphore("d_sem") as d_sem,
    ):
        # per-slot x semaphores: DMA queue completions are unordered across
        # chunks, so a single cumulative counter would be racy
        x_sems = [nc.alloc_semaphore(f"x_sem{s}") for s in range(XBUF)]
        dve_sem = nc.alloc_semaphore("dve_sem")
        rd_sem = nc.alloc_semaphore("rd_sem")   # intra-DVE RAW/WAR ordering
        w1_sems = [nc.alloc_semaphore(f"w1_sem{m}") for m in range(MPC)]
        w2_sems = [nc.alloc_semaphore(f"w2_sem{m}") for m in range(MPC)]

        def dma_x(sync, c):
            for kt in range(KT):
                sync.dma_start(
                    out=xs[:, c % XBUF, kt, :],
                    in_=xT_r[:, kt, c * CH:(c + 1) * CH],
                ).then_inc(x_sems[c % XBUF], 16)

        @block.sync
        def _(sync: bass.BassEngine):
            # interleave weight loads with early x chunks, ordered by need
            w1r = [w1[m].rearrange("(kt p) m2 -> p kt m2", p=128)
                   for m in range(MPC)]
            w2r = [w2[m].rearrange("(kt p) m2 -> p kt m2", p=128)
                   for m in range(MPC)]
            for kt in range(KT):
                sync.dma_start(out=w1s[0][:, kt], in_=w1r[0][:, kt]
                               ).then_inc(w1_sems[0], 16)
            sync.dma_start(out=b1s, in_=b1).then_inc(b1_sem, 16)
            dma_x(sync, 0)
            for kt in range(KT):
                sync.dma_start(out=w1s[1][:, kt], in_=w1r[1][:, kt]
                               ).then_inc(w1_sems[1], 16)
            for kt in range(KT):
                sync.dma_start(out=w2s[0][:, kt], in_=w2r[0][:, kt]
                               ).then_inc(w2_sems[0], 16)
            sync.dma_start(out=b2s, in_=b2).then_inc(b2_sem, 16)
            sync.dma_start(out=w3s, in_=w3).then_inc(w3_sem, 16)
            sync.dma_start(out=ones_s, in_=one).then_inc(w3_sem, 16)
            dma_x(sync, 1)
            for kt in range(KT):
                sync.dma_start(out=w2s[1][:, kt], in_=w2r[1][:, kt]
                               ).then_inc(w2_sems[1], 16)
            dma_x(sync, 2)
            dma_x(sync, 3)

            out_r = out.rearrange("m (nch ch) -> m nch ch", ch=CH)
            for c in range(XBUF, NCH):
                # x slot free once L1 of chunk c-XBUF fully consumed it
                sync.wait_ge(mm_sem, mmT[("l1", c - XBUF, MPC - 1, MT - 1)])
                dma_x(sync, c)
                # trailing store for chunk c-XBUF
                cs = c - XBUF
                sync.wait_ge(dve_sem, dve_cp(cs, MPC - 1))
                for m in range(MPC):
                    sync.dma_start(out=out_r[m:m + 1, cs],
                                   in_=osb[m][:, cs]).then_inc(d_sem, 16)

            for cs in range(NCH - XBUF, NCH):
                sync.wait_ge(dve_sem, dve_cp(cs, MPC - 1))
                for m in range(MPC):
                    sync.dma_start(out=out_r[m:m + 1, cs],
                                   in_=osb[m][:, cs]).then_inc(d_sem, 16)
            sync.wait_ge(d_sem, 16 * MPC * NCH)

        @block.vector
        def _(vector: bass.BassEngine):
            # DVE: (a) kt-reduction t_r = sum_kt w3[kt]*h2[kt] in exact fp32
            # (takes 3 of every 4 L3 matmuls off the PE, and is more accurate
            # than f32r products), (b) L3 psum -> osb copies.
            w3f = w3s.bitcast(f32)
            vector.wait_ge(w3_sem, 32)   # w3s + ones loaded
            for c in range(NCH):
                for m in range(MPC):
                    if c == NCH - 1:
                        break   # last chunk: PE computes L3 directly
                    # h2 ready; implies PE already read t_r(c-1, m) (its L3
                    # precedes this chunk's L2 in the PE stream)
                    vector.wait_ge(act_sem, act_r2(c, m, MT - 1))
                    h2f = h2[:, m].bitcast(f32)
                    # DVE does not self-interlock same-engine RAW/WAR;
                    # rd_sem orders the reduction chain explicitly
                    base = 6 * (MPC * c + m)
                    if base:
                        vector.wait_ge(rd_sem, base)
                    vector.tensor_scalar_mul(rA, h2f[:, 0, :], w3f[:, m, 0:1]
                                             ).then_inc(rd_sem, 1)
                    vector.tensor_scalar_mul(rB, h2f[:, 1, :], w3f[:, m, 1:2]
                                             ).then_inc(rd_sem, 1)
                    vector.wait_ge(rd_sem, base + 2)
                    vector.tensor_add(rC, rA, rB).then_inc(rd_sem, 1)
                    vector.wait_ge(rd_sem, base + 3)   # addC done before rA/rB reuse
                    vector.tensor_scalar_mul(rA, h2f[:, 2, :], w3f[:, m, 2:3]
                                             ).then_inc(rd_sem, 1)
                    vector.tensor_scalar_mul(rB, h2f[:, 3, :], w3f[:, m, 3:4]
                                             ).then_inc(rd_sem, 1)
                    vector.wait_ge(rd_sem, base + 5)
                    vector.tensor_add(rD, rA, rB).then_inc(rd_sem, 1)
                    vector.wait_ge(rd_sem, base + 6)
                    vector.tensor_add(t_r[:, m, :], rC, rD
                                      ).then_inc(dve_sem, 1)
                for m in range(MPC):
                    vector.wait_ge(mm_sem, mmT[("l3", c, m)])
                    vector.tensor_copy(
                        osb[m][0:1, c, :], psB[0:1, 2 * m + 1, :],
                    ).then_inc(dve_sem, 1)

        @block.tensor
        def _(tensor: bass.BassEngine):
            # warmup on uninitialized scratch: values are irrelevant, the psum
            # is overwritten (start=True) before any reader
            for i in range(N_WARM):
                tensor.matmul(psA[:, 0, :], scr[:, :128], scr[:, 128:],
                              start=True, stop=True, skip_group_check=True)

            def l3(c, m):
                # single ones-matmul over the DVE-reduced t_r; bank 2m+1 so
                # the osb copy only gates the SECOND L2 group of chunk c+1
                tensor.wait_ge(dve_sem, dve_red(c, m))
                tensor.matmul(
                    psB[0:1, 2 * m + 1, :], ones_s, t_r[:, m, :],
                    start=True, stop=True,
                ).then_inc(mm_sem, 1)

            for c in range(NCH):
                tensor.wait_ge(x_sems[c % XBUF], 64 * (c // XBUF + 1))
                # L1 both members
                for m in range(MPC):
                    if c == 0:
                        tensor.wait_ge(w1_sems[m], 64)
                    for mt in range(MT):
                        if mt >= 2:           # 2-bank rotation WAR
                            tensor.wait_ge(act_sem, act_r1(c, m, mt - 2))
                        elif c > 0:           # bank last used by c-1, mt+2
                            tensor.wait_ge(act_sem, act_r1(c - 1, m, mt + 2))
                        for kt in range(KT):
                            ins = tensor.matmul(
                                psA[:, 2 * m + mt % 2, :],
                                w1s[m][:, kt, mt * 128:(mt + 1) * 128],
                                xs[:, c % XBUF, kt, :],
                                start=(kt == 0), stop=(kt == KT - 1),
                            )
                        ins.then_inc(mm_sem, 1)
                # pipelined L3 of the previous chunk: its DVE reduction ran
                # while this chunk's L1 was on the PE
                if c >= 1:
                    if c == 1:
                        tensor.wait_ge(w3_sem, 32)
                    for m in range(MPC):
                        l3(c - 1, m)
                # L2 both members
                for m in range(MPC):
                    if c == 0:
                        tensor.wait_ge(w2_sems[m], 64)
                    tensor.wait_ge(act_sem, act_r1(c, m, MT - 1))  # h1 ready
                    for mt in range(MT):
                        if mt == 1 and c > 0:
                            # psB bank 2m+1 holds chunk c-1's L3 row until
                            # DVE copies it out
                            tensor.wait_ge(dve_sem, dve_cp(c - 1, m))
                        if mt >= 2:
                            tensor.wait_ge(act_sem, act_r2(c, m, mt - 2))
                        for kt in range(KT):
                            ins = tensor.matmul(
                                psB[:, 2 * m + mt % 2, :],
                                w2s[m][:, kt, mt * 128:(mt + 1) * 128],
                                h1[:, m, kt, :],
                                start=(kt == 0), stop=(kt == KT - 1),
                            )
                        ins.then_inc(mm_sem, 1)
            # tail: direct w3 matmuls for the last chunk -- avoids idling on
            # the serial DVE reduction after the final L2
            for m in range(MPC):
                tensor.wait_ge(act_sem, act_r2(NCH - 1, m, MT - 1))
                for kt in range(KT):
                    ins = tensor.matmul(
                        psB[0:1, 2 * m + 1, :],
                        w3s[:, m, kt:kt + 1],
                        h2[:, m, kt, :],
                        start=(kt == 0), stop=(kt == KT - 1),
                    )
                ins.then_inc(mm_sem, 1)

        @block.scalar
        def _(scalar: bass.BassEngine):
            Relu = bass.mybir.ActivationFunctionType.Relu
            scalar.wait_ge(b1_sem, 16)
            scalar.wait_ge(b2_sem, 16)
            for c in range(NCH):
                for m in range(MPC):
                    for mt in range(MT):
                        scalar.wait_ge(mm_sem, mmT[("l1", c, m, mt)])
                        scalar.activation(
                            h1[:, m, mt, :], psA[:, 2 * m + mt % 2, :], Relu,
                            bias=b1s[:, m, mt:mt + 1],
                        ).then_inc(act_sem, 1)
                for m in range(MPC):
                    for mt in range(MT):
                        scalar.wait_ge(mm_sem, mmT[("l2", c, m, mt)])
                        scalar.activation(
                            h2[:, m, mt, :], psB[:, 2 * m + mt % 2, :], Relu,
                            bias=b2s[:, m, mt:mt + 1],
                        ).then_inc(act_sem, 1)

    return nc


def get_nc():
    if "nc" not in _CACHE:
        _CACHE["nc"] = _build()
    return _CACHE["nc"]


def kernel(x, W1, b1, W2, b2, W3, b3):
    from concourse.bass_utils import run_bass_kernel_spmd

    nc = get_nc()
    xT = np.ascontiguousarray(np.asarray(x, dtype=np.float32).T)
    W1 = np.asarray(W1, dtype=np.float32)
    W2 = np.asarray(W2, dtype=np.float32)
    W3 = np.asarray(W3, dtype=np.float32)
    b1 = np.asarray(b1, dtype=np.float32)
    b2 = np.asarray(b2, dtype=np.float32)
    b3 = np.asarray(b3, dtype=np.float32)

    def feat_major(v):
        # [MPC, H] -> [128, MPC, H//128]: v[p, m, t] = v_in[m, t*128 + p]
        return np.ascontiguousarray(
            v.reshape(MPC, H // 128, 128).transpose(2, 0, 1))

    in_maps = []
    for c in range(N_CORES):
        s = slice(MPC * c, MPC * (c + 1))
        in_maps.append({
            "xT": xT,
            "w1": np.ascontiguousarray(W1[s]),
            "w2": np.ascontiguousarray(W2[s]),
            "w3": feat_major(W3[s, :, 0]),
            "b1": feat_major(b1[s]),
            "b2": feat_major(b2[s]),
            "one": np.ones((128, 1), dtype=np.float32),
        })

    res = run_bass_kernel_spmd(nc, in_maps, list(range(N_CORES)))
    out = np.concatenate([r["out"] for r in res.results], axis=0)  # [E, B]
    out = out + b3.reshape(E, 1)
    return out.reshape(E, B, 1).astype(np.float32)
